# revision 28
# baseline (speedup 1.0000x reference)
"""Multi-head attention on 8 TRN2 NeuronCores (Bass/Tile).

Sharding: core c handles batch b = c//2 and query-half h = c%2 (1024 query
tokens), all 16 heads. K/V projections are per-batch and duplicated across
the two cores sharing a batch; no cross-core communication (pair-AllGather
dedup was prototyped but the cost model prices a collective at 15us +
total_bytes/40GB/s, which never pays for the ~15-30us of PE it saves).

Design notes (v9, evolved from v3):
- All matmul operands bf16 (PSUM fp32). Keys compacted on host via the 0/1
  mask; pad keys killed by a -1e9 per-partition bias folded into Exp.
- V-proj bias folded into output bias host-side (bo' = bo + Wo @ bv).
- Energy computed transposed ([key, query] tiles); AV accumulates
  out[q, 65] per head with a ones column carrying the softmax denominator.
- K-projection free dims trimmed to the actual key count (ntrim); KT pad
  columns memset to zero. kt input tile also trimmed.
- Input DMAs split across the two HWDGE queues: SP carries qt/wq/kt/wk +
  transposes + output; the Activation queue carries vt/wv/wo. (The Pool
  SWDGE path was tried and CORRUPTS data under concurrent consumers --
  it races the cross-engine readers; the Act queue is a proper HWDGE
  path and is safe.)
- Out-projection split in three k-chunk groups per (m, n0): A=k0-3 (ready
  once OTs[0..3] final, after unit 9), B=k4-6 (after unit 15), C=k7
  (tail), accumulating into bf16 SBUF tiles (yac). m=7 runs entirely at
  the tail (its yac slot didn't fit in SBUF). Output yT is bf16 (host
  converts to f32); total extra error ~0.2-0.3% rms, well inside 2e-2.
- Attention phase is PE-bound (PE busy ~207us > Act exp stream ~151us).
  Q-proj runs as two 4-kk passes so pass 0 starts when the first qt/wq
  half lands. Phase 1 carries Q m0 per-kk + pass0 all m + pass1 m0 +
  K m0; the rest (Q pass1 m1-7, K m1-7, V t0-8, out-proj groups A/B)
  drain as a deadline-tagged filler deque, one chunk per two kt-steps;
  an item is force-emitted at the top of the unit that first reads its
  output (dependency tracking is program-order directional: a read
  emitted before its writer races it, which shows up as NaN on HW while
  the timeline simulator still passes).
- The PE p-state model punishes sparse matmul streams: a gap resets the
  ramp and subsequent matmuls run at half clock for ~3us. Prefer dense
  chunks (>=4 matmuls per psF tile, PE-bound vs the DVE consumer) over
  per-kk trickles.
"""

import sys

sys.path.insert(0, "/opt/trn_rl_repo")

from contextlib import ExitStack

import ml_dtypes
import numpy as np

import concourse.bass as bass  # noqa: F401
import concourse.tile as tile
from concourse import bacc, mybir
from concourse.bass_utils import run_bass_kernel_spmd

E = 1024          # embed dim
HEADS = 16
HD = 64           # head dim
B = 4
S = 2048
NCORES = 8
Q = (B * S) // NCORES  # query tokens per core
EC = E // 128     # embed chunks of 128
F32 = mybir.dt.float32
BF16 = mybir.dt.bfloat16
BF16NP = ml_dtypes.bfloat16


def build_program(Kpad, ntrim):
    """Per-core Bass program (identical on all 8 cores)."""
    KTn = Kpad // 128
    nc = bacc.Bacc("TRN2", target_bir_lowering=False, debug=False,
                   num_devices=NCORES, dynamic_dma_scratch_size=2048)

    qT = nc.dram_tensor("qT", [E, Q], BF16, kind="ExternalInput").ap()
    kT = nc.dram_tensor("kT", [E, Kpad], BF16, kind="ExternalInput").ap()
    vT = nc.dram_tensor("vT", [E, Kpad], BF16, kind="ExternalInput").ap()
    wqT = nc.dram_tensor("wqT", [E, E], BF16, kind="ExternalInput").ap()
    wkT = nc.dram_tensor("wkT", [E, E], BF16, kind="ExternalInput").ap()
    wvT = nc.dram_tensor("wvT", [E, E], BF16, kind="ExternalInput").ap()
    woT = nc.dram_tensor("woT", [E, E], BF16, kind="ExternalInput").ap()
    bq2 = nc.dram_tensor("bq2", [128, EC], F32, kind="ExternalInput").ap()
    bk2 = nc.dram_tensor("bk2", [128, EC], F32, kind="ExternalInput").ap()
    bo2 = nc.dram_tensor("bo2", [128, EC], F32, kind="ExternalInput").ap()
    mb = nc.dram_tensor("mb", [128, KTn], F32, kind="ExternalInput").ap()
    yT = nc.dram_tensor("yT", [E, Q], BF16, kind="ExternalOutput").ap()

    with tile.TileContext(nc) as tc, ExitStack() as ctx:
        sml = ctx.enter_context(tc.tile_pool(name="sml", bufs=1))
        big = ctx.enter_context(tc.tile_pool(name="big", bufs=1))

        # ---- PSUM pools: psE 2x[128,1024]=4 banks, psA 2x1=2, psF 2x1=2
        psE = ctx.enter_context(tc.tile_pool(name="psE", bufs=2, space="PSUM"))
        psA = ctx.enter_context(tc.tile_pool(name="psA", bufs=1, space="PSUM"))
        psF = ctx.enter_context(tc.tile_pool(name="psF", bufs=2, space="PSUM"))

        inp = ctx.enter_context(tc.tile_pool(name="inp", bufs=1))
        pp = ctx.enter_context(tc.tile_pool(name="pp", bufs=1))
        nrm = ctx.enter_context(tc.tile_pool(name="nrm", bufs=2))
        # vt/wv free mid-attention; inpV sits atop the pool stack so its
        # SBUF can be reused for wo/yac/yt.
        inpV_ctx = ExitStack()
        inpV = inpV_ctx.enter_context(tc.tile_pool(name="inpV", bufs=1))

        # ---- big input DMAs (single SP/HWDGE queue, consumption order) ---
        qt_t, wq_t = [], []
        qv = qT[:].rearrange("(c p) q -> p c q", p=128)
        wqv = wqT[:].rearrange("(c p) e -> p c e", p=128)
        bq_s = sml.tile([128, EC], F32, name="bq_s")
        bk_s = sml.tile([128, EC], F32, name="bk_s")
        bo_s = sml.tile([128, EC], F32, name="bo_s")
        mb_s = sml.tile([128, KTn], F32, name="mb_s")
        for h in (0, 1):
            t = inp.tile([128, 4, Q], BF16, name=f"qt{h}")
            w = inp.tile([128, 4, E], BF16, name=f"wq{h}")
            for kk in range(4):
                nc.sync.dma_start(t[:, kk, :], qv[:, 4 * h + kk, :])
                nc.sync.dma_start(w[:, kk, :], wqv[:, 4 * h + kk, :])
                if h == 0 and kk == 1:
                    nc.sync.dma_start(bq_s[:], bq2[:])
                    nc.sync.dma_start(bk_s[:], bk2[:])
                    nc.sync.dma_start(bo_s[:], bo2[:])
                    nc.sync.dma_start(mb_s[:], mb[:])
            qt_t.append(t)
            wq_t.append(w)
        kt_t = inp.tile([128, EC, ntrim], BF16, name="kt")
        kv4 = kT[:].rearrange("(c p) k -> p c k", p=128)
        nc.sync.dma_start(kt_t[:, 0:4, :], kv4[:, 0:4, 0:ntrim])
        nc.sync.dma_start(kt_t[:, 4:8, :], kv4[:, 4:8, 0:ntrim])
        wk_t = inp.tile([128, EC, E], BF16, name="wk")
        wkv = wkT[:].rearrange("(c p) e -> p c e", p=128)
        nc.sync.dma_start(wk_t[:, 0:4, :], wkv[:, 0:4, :])
        nc.sync.dma_start(wk_t[:, 4:8, :], wkv[:, 4:8, :])
        vt_t = inpV.tile([128, EC, Kpad], BF16, name="vt")
        vv4 = vT[:].rearrange("(c p) k -> p c k", p=128)
        wv_t = inpV.tile([128, EC, E], BF16, name="wv")
        wvv = wvT[:].rearrange("(c p) e -> p c e", p=128)
        # The Act sequencer runs from t=0 regardless of emission position;
        # gate its DMA stream on an SP-loaded tile so vt/wv transfers don't
        # steal the DMA engines from unit-0's qt/wq feed.
        gate = sml.tile([128, 1], F32, name="actgate")
        nc.scalar.mul(gate[:], bq_s[:, 0:1], 1.0)
        for c0 in (0, 2, 4, 6):
            nc.scalar.dma_start(vt_t[:, c0:c0 + 2, :], vv4[:, c0:c0 + 2, :])
            nc.scalar.dma_start(wv_t[:, c0:c0 + 2, :], wvv[:, c0:c0 + 2, :])

        # ---- persistent SBUF tensors -------------------------------------
        QTs = [big.tile([128, Q], BF16, name=f"QT{m}") for m in range(EC)]
        KTs = [big.tile([128, Kpad], BF16, name=f"KT{m}") for m in range(EC)]
        VVs = [big.tile([128, HEADS * 65], BF16, name=f"VV{t}")
               for t in range(KTn)]
        OTs = [big.tile([128, Q], BF16, name=f"OT{m}") for m in range(EC)]
        # ones columns of the V tiles, written once before anything reads VV
        for t in range(KTn):
            vv3 = VVs[t][:].rearrange("p (h e) -> p h e", e=65)
            nc.vector.memset(vv3[:, :, 64:65], 1.0)

        # K-proj free-dim chunks, trimmed to ntrim
        kchunks = []
        for n0 in range(0, ntrim, 512):
            kchunks.append((n0, min(512, ntrim - n0)))

        # ---- projection emitters -----------------------------------------
        def q_pass(m, half):
            """Half-contraction Q-proj pass: kk in [half*4, half*4+4).
            Pass 0 writes QTs[m] (with bias); pass 1 accumulates."""
            for n0 in (0, 512):
                q_chunk(m, n0, half * 4, 4, half == 0)

        def q_chunk(m, n0, k0=0, nk=EC, first=True):
            ps = psF.tile([128, 512], F32, tag="f", name=f"psq{m}_{n0}_{k0}")
            for kk in range(k0, k0 + nk):
                nc.tensor.matmul(
                    ps[:], wq_t[kk // 4][:, kk % 4, m * 128:(m + 1) * 128],
                    qt_t[kk // 4][:, kk % 4, n0:n0 + 512],
                    start=(kk == k0), stop=(kk == k0 + nk - 1))
            if first:
                nc.vector.tensor_scalar_add(
                    QTs[m][:, n0:n0 + 512], ps[:], bq_s[:, m:m + 1])
            else:
                with nc.allow_low_precision(reason="bf16 proj accum"):
                    nc.vector.tensor_add(
                        QTs[m][:, n0:n0 + 512], QTs[m][:, n0:n0 + 512], ps[:])

        def k_chunk(m, n0, nn, last):
            ps = psF.tile([128, 512], F32, tag="f", name=f"psk{m}_{n0}")
            for kk in range(EC):
                nc.tensor.matmul(
                    ps[:, 0:nn], wk_t[:, kk, m * 128:(m + 1) * 128],
                    kt_t[:, kk, n0:n0 + nn],
                    start=(kk == 0), stop=(kk == EC - 1))
            nc.vector.tensor_scalar_add(
                KTs[m][:, n0:n0 + nn], ps[:, 0:nn], bk_s[:, m:m + 1])
            if last and ntrim < Kpad:
                nc.vector.memset(KTs[m][:, ntrim:Kpad], 0.0)

        def k_slot(m):
            for ci, (n0, nn) in enumerate(kchunks):
                k_chunk(m, n0, nn, ci == len(kchunks) - 1)

        def v_chunk(t, half):
            """V-proj chunk: heads half*8..half*8+8 of key tile t."""
            n0 = half * 512
            ps = psF.tile([128, 512], F32, tag="f", name=f"psv{t}_{half}")
            for kk in range(EC):
                nc.tensor.matmul(
                    ps[:], vt_t[:, kk, t * 128:(t + 1) * 128],
                    wv_t[:, kk, n0:n0 + 512],
                    start=(kk == 0), stop=(kk == EC - 1))
            vv3 = VVs[t][:].rearrange("p (h e) -> p h e", e=65)
            ps3 = ps[:].rearrange("p (h d) -> p h d", d=64)
            nc.vector.tensor_copy(vv3[:, half * 8:half * 8 + 8, 0:64], ps3[:])

        # ---- phase 1 emission -------------------------------------------
        # Q pass 0 (kk 0-3) for all m as the first qt/wq half arrives;
        # pass 1 for m=0, then K m0 -> attention unit 0 can start.
        for k in range(4):
            for n0 in (0, 512):
                q_chunk(0, n0, k, 1, k == 0)
        for m in range(1, EC):
            q_pass(m, 0)
        q_pass(0, 1)
        k_slot(0)

        # ---- out-projection emitters -------------------------------------
        wo_t = [None]
        yac = []   # bf16 accumulators for m=0..6, allocated in wop at u==4
        yts = []   # two rotating bf16 output staging tiles

        def o_chunk(m, n0, k0, nk):
            ps = psF.tile([128, 512], F32, tag="f", name=f"pso{m}_{n0}_{k0}")
            for k in range(k0, k0 + nk):
                nc.tensor.matmul(
                    ps[:], wo_t[0][:, k, m * 128:(m + 1) * 128],
                    OTs[k][:, n0:n0 + 512],
                    start=(k == k0), stop=(k == k0 + nk - 1))
            if k0 == 0 and nk < EC:
                with nc.allow_low_precision(reason="bf16 out-proj accum"):
                    nc.vector.tensor_copy(yac[m][:, n0:n0 + 512], ps[:])
            elif k0 + nk < EC:
                with nc.allow_low_precision(reason="bf16 out-proj accum"):
                    nc.vector.tensor_add(yac[m][:, n0:n0 + 512],
                                         yac[m][:, n0:n0 + 512], ps[:])
            elif nk == EC:  # m=7: whole contraction in one chunk
                yt = yts[n0 // 512]
                with nc.allow_low_precision(reason="bf16 output"):
                    nc.vector.tensor_scalar_add(
                        yt[:], ps[:], bo_s[:, m:m + 1])
                nc.sync.dma_start(yT[m * 128:(m + 1) * 128, n0:n0 + 512],
                                  yt[:])
            else:
                # finish in place: yac[m] slice becomes the output staging
                dst = yac[m][:, n0:n0 + 512]
                with nc.allow_low_precision(reason="bf16 output"):
                    nc.vector.scalar_tensor_tensor(
                        dst, ps[:], bo_s[:, m:m + 1], dst,
                        op0=mybir.AluOpType.add, op1=mybir.AluOpType.add)
                nc.sync.dma_start(yT[m * 128:(m + 1) * 128, n0:n0 + 512],
                                  dst)

        # ---- attention: units of (head pair j, query half qh) ------------
        def emit_av_kt(st, kt):
            j, ptiles, avs = st["j"], st["pt"], st["avs"]
            for pair in (0, 1):
                av = avs[pair]
                for qi in (0, 1):
                    for hh in (0, 1):
                        idx = kt * 4 + qi * 2 + hh
                        q0 = hh * 512 + (pair * 2 + qi) * 128
                        nc.tensor.matmul(
                            av[:, (qi * 2 + hh) * 65:(qi * 2 + hh + 1) * 65],
                            ptiles[kt][:, q0:q0 + 128],
                            VVs[kt][:, (2 * j + hh) * 65:(2 * j + hh + 1) * 65],
                            start=(idx == 0), stop=(idx == KTn * 4 - 1),
                            skip_group_check=True)

        def finalize_av(st):
            j, qh, avs, oj = st["j"], st["qh"], st["avs"], st["oj"]
            for pair in (0, 1):
                av = avs[pair]
                av3 = av[:].rearrange("p (x c) -> p x c", c=65)
                rc = nrm.tile([128, 4], F32, tag="rc",
                              name=f"rc{j}_{qh}_{pair}")
                nc.vector.reciprocal(
                    rc[:].rearrange("p (a b) -> p a b", b=1), av3[:, :, 64:65])
                for qi in (0, 1):
                    ql = pair * 2 + qi
                    qc = qh * 4 + ql
                    for hh in (0, 1):
                        i = qi * 2 + hh
                        nc.vector.tensor_scalar_mul(
                            oj[:, ql, hh * 64:hh * 64 + 64],
                            av[:, i * 65:i * 65 + 64], rc[:, i:i + 1])
                    nc.sync.dma_start_transpose(
                        OTs[j][:, qc * 128:(qc + 1) * 128], oj[:, ql, :])

        HIPRI = 1 << 20
        units = [(j, qh) for j in range(EC) for qh in (0, 1)]

        # Filler inventory: (emission_deadline_unit, closure). Paced pops
        # drain one item per two kt-steps; any item whose deadline arrives
        # is force-emitted at the top of that unit (emission must precede
        # the first reader -- dependency tracking is program-order
        # directional). List order is psF-execution order: m1 fillers
        # before the V block so attention unit 2 isn't gated on V-proj.
        fillers = [
            (2, lambda: q_pass(1, 1)),
            (2, lambda: k_slot(1)),
        ]
        for t in range(KTn):
            fillers.append((1, lambda t=t: v_chunk(t, 0)))
            fillers.append((1, lambda t=t: v_chunk(t, 1)))
        for m in range(2, EC):
            fillers.append((2 * m, lambda m=m: q_pass(m, 1)))
            fillers.append((2 * m, lambda m=m: k_slot(m)))
        oA = [lambda m=m, n0=n0: o_chunk(m, n0, 0, 4)
              for m in range(EC - 1) for n0 in (0, 512)]
        oB = [lambda m=m, n0=n0: o_chunk(m, n0, 4, 3)
              for m in range(EC - 1) for n0 in (0, 512)]
        oC = [lambda m=m, n0=n0: o_chunk(m, n0, 7, 1)
              for m in range(EC - 1) for n0 in (0, 512)]
        oC += [lambda n0=n0: o_chunk(EC - 1, n0, 0, EC) for n0 in (0, 512)]

        prev = None
        for u, (j, qh) in enumerate(units):
            ptiles = []
            if u == 9:
                fillers.extend((10 ** 9, c) for c in oA)
            if u == 15:
                fillers.extend((10 ** 9, c) for c in oB)
            due = [f for f in fillers if f[0] <= u]
            if due:
                fillers = [f for f in fillers if f[0] > u]
                for _, c in due:
                    c()
            if u == 4:
                # vt/wv no longer needed; reuse the SBUF for wo/yac/yt.
                inpV_ctx.close()
                wo_pool = ctx.enter_context(tc.tile_pool(name="wop", bufs=1))
                wo_t[0] = wo_pool.tile([128, EC, E], BF16, name="wo")
                wov = woT[:].rearrange("(c p) e -> p c e", p=128)
                nc.scalar.dma_start(wo_t[0][:, 0:4, :], wov[:, 0:4, :])
                nc.scalar.dma_start(wo_t[0][:, 4:8, :], wov[:, 4:8, :])
                yac.extend(wo_pool.tile([128, Q], BF16, name=f"yac{m}")
                           for m in range(EC - 1))
                yts.extend(wo_pool.tile([128, 512], BF16, name=f"yt{i}")
                           for i in range(2))
            for kt in range(KTn):
                with tc.high_priority(offset=HIPRI):
                    pe = psE.tile([128, 1024], F32, tag="e",
                                  name=f"pe{j}_{qh}_{kt}")
                    for hh in (0, 1):
                        off = hh * 64
                        nc.tensor.matmul(
                            pe[:, hh * 512:hh * 512 + 512],
                            KTs[j][off:off + 64, kt * 128:(kt + 1) * 128],
                            QTs[j][off:off + 64, qh * 512:qh * 512 + 512])
                    pt = pp.tile([128, 1024], BF16, tag=f"P{qh}_{kt}",
                                 name=f"pt{j}_{qh}_{kt}")
                    nc.scalar.activation(
                        pt[:], pe[:], mybir.ActivationFunctionType.Exp,
                        bias=mb_s[:, kt:kt + 1], scale=0.125)
                    ptiles.append(pt)
                    if prev is not None:
                        if kt == 0:
                            prev["avs"] = [
                                psA.tile([128, 260], F32, tag=f"a{pr}",
                                         name=f"av{prev['j']}_{prev['qh']}_{pr}")
                                for pr in (0, 1)]
                        emit_av_kt(prev, kt)
                if kt % 2 == 1 and fillers:
                    fillers.pop(0)[1]()
            if prev is not None:
                with tc.high_priority(offset=HIPRI):
                    finalize_av(prev)
            prev = dict(j=j, qh=qh, pt=ptiles, avs=None,
                        oj=nrm.tile([128, 4, 128], BF16, tag="oj",
                                    name=f"oj{j}_{qh}"))
        with tc.high_priority(offset=HIPRI):
            prev["avs"] = [psA.tile([128, 260], F32, tag=f"a{pr}",
                                    name=f"av_last_{pr}") for pr in (0, 1)]
            for kt in range(KTn):
                emit_av_kt(prev, kt)
            finalize_av(prev)
        while fillers:
            fillers.pop(0)[1]()
        for c in oC:
            c()

    nc.compile()
    return nc


_PROG_CACHE = {}


def _get_program(Kpad, ntrim):
    key = (Kpad, ntrim)
    if key not in _PROG_CACHE:
        _PROG_CACHE[key] = build_program(Kpad, ntrim)
    return _PROG_CACHE[key]


def prepare_inputs(query, keys, values, mask, Wq, bq, Wk, bk, Wv, bv, Wo, bo):
    """Host-side sharding/layout prep. Returns (Kpad, ntrim, in_maps)."""
    f32 = np.float32
    query = np.asarray(query, f32)
    keys = np.asarray(keys, f32)
    values = np.asarray(values, f32)
    mask = np.asarray(mask)

    idxs = [np.nonzero(mask[b] != 0)[0] for b in range(B)]
    nmax = max(len(i) for i in idxs)
    Kpad = max(256, ((max(nmax, 1) + 127) // 128) * 128)
    KTn = Kpad // 128
    ntrim = min(Kpad, ((max(nmax, 1) + 3) // 4) * 4)

    kTb = np.zeros((B, E, Kpad), BF16NP)
    vTb = np.zeros((B, E, Kpad), BF16NP)
    mbb = np.full((B, Kpad), -1e9, f32)
    for b in range(B):
        n = len(idxs[b])
        kTb[b, :, :n] = keys[b][idxs[b]].T.astype(BF16NP)
        vTb[b, :, :n] = values[b][idxs[b]].T.astype(BF16NP)
        mbb[b, :n] = 0.0
    mb2 = np.ascontiguousarray(mbb.reshape(B, KTn, 128).transpose(0, 2, 1))

    WqT = np.ascontiguousarray(np.asarray(Wq, f32).T.astype(BF16NP))
    WkT = np.ascontiguousarray(np.asarray(Wk, f32).T.astype(BF16NP))
    WvT = np.ascontiguousarray(np.asarray(Wv, f32).T.astype(BF16NP))
    WoT = np.ascontiguousarray(np.asarray(Wo, f32).T.astype(BF16NP))
    bq2 = np.ascontiguousarray(np.asarray(bq, f32).reshape(EC, 128).T)
    bk2 = np.ascontiguousarray(np.asarray(bk, f32).reshape(EC, 128).T)
    # fold V bias through the output projection: y += (Wo @ bv + bo)
    bo_f = np.asarray(bo, f32) + np.asarray(Wo, f32) @ np.asarray(bv, f32)
    bo2 = np.ascontiguousarray(bo_f.reshape(EC, 128).T)

    in_maps = []
    for c in range(NCORES):
        b, h = c // 2, c % 2
        in_maps.append(dict(
            qT=np.ascontiguousarray(
                query[b, h * Q:(h + 1) * Q, :].T.astype(BF16NP)),
            kT=kTb[b], vT=vTb[b], mb=mb2[b],
            wqT=WqT, wkT=WkT, wvT=WvT, woT=WoT,
            bq2=bq2, bk2=bk2, bo2=bo2,
        ))
    return Kpad, ntrim, in_maps


def kernel(query, keys, values, mask, Wq, bq, Wk, bk, Wv, bv, Wo, bo):
    Kpad, ntrim, in_maps = prepare_inputs(query, keys, values, mask,
                                          Wq, bq, Wk, bk, Wv, bv, Wo, bo)
    nc = _get_program(Kpad, ntrim)
    res = run_bass_kernel_spmd(nc, in_maps, list(range(NCORES)))
    out = np.empty((B, S, E), np.float32)
    for c in range(NCORES):
        b, h = c // 2, c % 2
        out[b, h * Q:(h + 1) * Q, :] = \
            res.results[c]["yT"].T.astype(np.float32)
    return out


# revision 30
# speedup vs baseline: 1.0175x; 1.0175x over previous
"""Multi-head attention on 8 TRN2 NeuronCores (Bass/Tile).

Sharding: core c handles batch b = c//2 and query-half h = c%2 (1024 query
tokens), all 16 heads. K/V projections are per-batch and duplicated across
the two cores sharing a batch; no cross-core communication (pair-AllGather
dedup was prototyped but the cost model prices a collective at 15us +
total_bytes/40GB/s, which never pays for the ~15-30us of PE it saves).

Design notes (v9, evolved from v3):
- All matmul operands bf16 (PSUM fp32). Keys compacted on host via the 0/1
  mask; pad keys killed by a -1e9 per-partition bias folded into Exp.
- V-proj bias folded into output bias host-side (bo' = bo + Wo @ bv).
- Energy computed transposed ([key, query] tiles); AV accumulates
  out[q, 65] per head with a ones column carrying the softmax denominator.
- K-projection free dims trimmed to the actual key count (ntrim); KT pad
  columns memset to zero. kt input tile also trimmed.
- Input DMAs split across the two HWDGE queues: SP carries qt/wq/kt/wk +
  transposes + output; the Activation queue carries vt/wv/wo. (The Pool
  SWDGE path was tried and CORRUPTS data under concurrent consumers --
  it races the cross-engine readers; the Act queue is a proper HWDGE
  path and is safe.)
- Out-projection split in three k-chunk groups per (m, n0): A=k0-3 (ready
  once OTs[0..3] final, after unit 9), B=k4-6 (after unit 15), C=k7
  (tail), accumulating into bf16 SBUF tiles (yac). m=7 runs entirely at
  the tail (its yac slot didn't fit in SBUF). Output yT is bf16 (host
  converts to f32); total extra error ~0.2-0.3% rms, well inside 2e-2.
- Attention phase is PE-bound (PE busy ~207us > Act exp stream ~151us).
  Q-proj runs as two 4-kk passes so pass 0 starts when the first qt/wq
  half lands. Phase 1 carries Q m0 per-kk + pass0 all m + pass1 m0 +
  K m0; the rest (Q pass1 m1-7, K m1-7, V t0-8, out-proj groups A/B)
  drain as a deadline-tagged filler deque, one chunk per two kt-steps;
  an item is force-emitted at the top of the unit that first reads its
  output (dependency tracking is program-order directional: a read
  emitted before its writer races it, which shows up as NaN on HW while
  the timeline simulator still passes).
- The PE p-state model punishes sparse matmul streams: a gap resets the
  ramp and subsequent matmuls run at half clock for ~3us. Prefer dense
  chunks (>=4 matmuls per psF tile, PE-bound vs the DVE consumer) over
  per-kk trickles.
"""

import sys

sys.path.insert(0, "/opt/trn_rl_repo")

from contextlib import ExitStack

import ml_dtypes
import numpy as np

import concourse.bass as bass  # noqa: F401
import concourse.tile as tile
from concourse import bacc, mybir
from concourse.bass_utils import run_bass_kernel_spmd

E = 1024          # embed dim
HEADS = 16
HD = 64           # head dim
B = 4
S = 2048
NCORES = 8
Q = (B * S) // NCORES  # query tokens per core
EC = E // 128     # embed chunks of 128
F32 = mybir.dt.float32
BF16 = mybir.dt.bfloat16
BF16NP = ml_dtypes.bfloat16


def build_program(Kpad, ntrim):
    """Per-core Bass program (identical on all 8 cores)."""
    KTn = Kpad // 128
    nc = bacc.Bacc("TRN2", target_bir_lowering=False, debug=False,
                   num_devices=NCORES, dynamic_dma_scratch_size=2048)

    qT = nc.dram_tensor("qT", [E, Q], BF16, kind="ExternalInput").ap()
    kT = nc.dram_tensor("kT", [E, Kpad], BF16, kind="ExternalInput").ap()
    vT = nc.dram_tensor("vT", [E, Kpad], BF16, kind="ExternalInput").ap()
    wqT = nc.dram_tensor("wqT", [E, E], BF16, kind="ExternalInput").ap()
    wkT = nc.dram_tensor("wkT", [E, E], BF16, kind="ExternalInput").ap()
    wvT = nc.dram_tensor("wvT", [E, E], BF16, kind="ExternalInput").ap()
    woT = nc.dram_tensor("woT", [E, E], BF16, kind="ExternalInput").ap()
    bq2 = nc.dram_tensor("bq2", [128, EC], F32, kind="ExternalInput").ap()
    bk2 = nc.dram_tensor("bk2", [128, EC], F32, kind="ExternalInput").ap()
    bo2 = nc.dram_tensor("bo2", [128, EC], F32, kind="ExternalInput").ap()
    mb = nc.dram_tensor("mb", [128, KTn], F32, kind="ExternalInput").ap()
    yT = nc.dram_tensor("yT", [E, Q], BF16, kind="ExternalOutput").ap()

    with tile.TileContext(nc) as tc, ExitStack() as ctx:
        sml = ctx.enter_context(tc.tile_pool(name="sml", bufs=1))
        big = ctx.enter_context(tc.tile_pool(name="big", bufs=1))

        # ---- PSUM pools: psE 2x[128,1024]=4 banks, psA 2x1=2, psF 2x1=2
        psE = ctx.enter_context(tc.tile_pool(name="psE", bufs=2, space="PSUM"))
        psA = ctx.enter_context(tc.tile_pool(name="psA", bufs=1, space="PSUM"))
        psF = ctx.enter_context(tc.tile_pool(name="psF", bufs=2, space="PSUM"))

        inp = ctx.enter_context(tc.tile_pool(name="inp", bufs=1))
        pp = ctx.enter_context(tc.tile_pool(name="pp", bufs=1))
        nrm = ctx.enter_context(tc.tile_pool(name="nrm", bufs=2))
        # vt/wv free mid-attention; inpV sits atop the pool stack so its
        # SBUF can be reused for wo/yac/yt.
        inpV_ctx = ExitStack()
        inpV = inpV_ctx.enter_context(tc.tile_pool(name="inpV", bufs=1))

        # ---- big input DMAs (single SP/HWDGE queue, consumption order) ---
        qt_t, wq_t = [], []
        qv = qT[:].rearrange("(c p) q -> p c q", p=128)
        wqv = wqT[:].rearrange("(c p) e -> p c e", p=128)
        bq_s = sml.tile([128, EC], F32, name="bq_s")
        bk_s = sml.tile([128, EC], F32, name="bk_s")
        bo_s = sml.tile([128, EC], F32, name="bo_s")
        mb_s = sml.tile([128, KTn], F32, name="mb_s")
        for h in (0, 1):
            t = inp.tile([128, 4, Q], BF16, name=f"qt{h}")
            w = inp.tile([128, 4, E], BF16, name=f"wq{h}")
            for kk in range(4):
                nc.sync.dma_start(t[:, kk, :], qv[:, 4 * h + kk, :])
                nc.sync.dma_start(w[:, kk, :], wqv[:, 4 * h + kk, :])
                if h == 0 and kk == 1:
                    nc.sync.dma_start(bq_s[:], bq2[:])
                    nc.sync.dma_start(bk_s[:], bk2[:])
                    nc.sync.dma_start(bo_s[:], bo2[:])
                    nc.sync.dma_start(mb_s[:], mb[:])
            qt_t.append(t)
            wq_t.append(w)
        kt_t = inp.tile([128, EC, ntrim], BF16, name="kt")
        kv4 = kT[:].rearrange("(c p) k -> p c k", p=128)
        nc.sync.dma_start(kt_t[:, 0:4, :], kv4[:, 0:4, 0:ntrim])
        nc.sync.dma_start(kt_t[:, 4:8, :], kv4[:, 4:8, 0:ntrim])
        wk_t = inp.tile([128, EC, E], BF16, name="wk")
        wkv = wkT[:].rearrange("(c p) e -> p c e", p=128)
        nc.sync.dma_start(wk_t[:, 0:4, :], wkv[:, 0:4, :])
        nc.sync.dma_start(wk_t[:, 4:8, :], wkv[:, 4:8, :])
        vt_t = inpV.tile([128, EC, Kpad], BF16, name="vt")
        vv4 = vT[:].rearrange("(c p) k -> p c k", p=128)
        wv_t = inpV.tile([128, EC, E], BF16, name="wv")
        wvv = wvT[:].rearrange("(c p) e -> p c e", p=128)
        for c0 in (0, 2, 4, 6):
            nc.scalar.dma_start(vt_t[:, c0:c0 + 2, :], vv4[:, c0:c0 + 2, :])
            nc.scalar.dma_start(wv_t[:, c0:c0 + 2, :], wvv[:, c0:c0 + 2, :])

        # ---- persistent SBUF tensors -------------------------------------
        QTs = [big.tile([128, Q], BF16, name=f"QT{m}") for m in range(EC)]
        KTs = [big.tile([128, Kpad], BF16, name=f"KT{m}") for m in range(EC)]
        VVs = [big.tile([128, HEADS * 65], BF16, name=f"VV{t}")
               for t in range(KTn)]
        OTs = [big.tile([128, Q], BF16, name=f"OT{m}") for m in range(EC)]
        # ones columns of the V tiles, written once before anything reads VV
        for t in range(KTn):
            vv3 = VVs[t][:].rearrange("p (h e) -> p h e", e=65)
            nc.vector.memset(vv3[:, :, 64:65], 1.0)

        # K-proj free-dim chunks, trimmed to ntrim
        kchunks = []
        for n0 in range(0, ntrim, 512):
            kchunks.append((n0, min(512, ntrim - n0)))

        # ---- projection emitters -----------------------------------------
        def q_pass(m, half):
            """Half-contraction Q-proj pass: kk in [half*4, half*4+4).
            Pass 0 writes QTs[m] (with bias); pass 1 accumulates."""
            for n0 in (0, 512):
                q_chunk(m, n0, half * 4, 4, half == 0)

        def q_chunk(m, n0, k0=0, nk=EC, first=True):
            ps = psF.tile([128, 512], F32, tag="f", name=f"psq{m}_{n0}_{k0}")
            for kk in range(k0, k0 + nk):
                nc.tensor.matmul(
                    ps[:], wq_t[kk // 4][:, kk % 4, m * 128:(m + 1) * 128],
                    qt_t[kk // 4][:, kk % 4, n0:n0 + 512],
                    start=(kk == k0), stop=(kk == k0 + nk - 1))
            if first:
                nc.vector.tensor_scalar_add(
                    QTs[m][:, n0:n0 + 512], ps[:], bq_s[:, m:m + 1])
            else:
                with nc.allow_low_precision(reason="bf16 proj accum"):
                    nc.vector.tensor_add(
                        QTs[m][:, n0:n0 + 512], QTs[m][:, n0:n0 + 512], ps[:])

        def k_chunk(m, n0, nn, last):
            ps = psF.tile([128, 512], F32, tag="f", name=f"psk{m}_{n0}")
            for kk in range(EC):
                nc.tensor.matmul(
                    ps[:, 0:nn], wk_t[:, kk, m * 128:(m + 1) * 128],
                    kt_t[:, kk, n0:n0 + nn],
                    start=(kk == 0), stop=(kk == EC - 1))
            nc.vector.tensor_scalar_add(
                KTs[m][:, n0:n0 + nn], ps[:, 0:nn], bk_s[:, m:m + 1])
            if last and ntrim < Kpad:
                nc.vector.memset(KTs[m][:, ntrim:Kpad], 0.0)

        def k_slot(m):
            for ci, (n0, nn) in enumerate(kchunks):
                k_chunk(m, n0, nn, ci == len(kchunks) - 1)

        def v_chunk(t, half):
            """V-proj chunk: heads half*8..half*8+8 of key tile t."""
            n0 = half * 512
            ps = psF.tile([128, 512], F32, tag="f", name=f"psv{t}_{half}")
            for kk in range(EC):
                nc.tensor.matmul(
                    ps[:], vt_t[:, kk, t * 128:(t + 1) * 128],
                    wv_t[:, kk, n0:n0 + 512],
                    start=(kk == 0), stop=(kk == EC - 1))
            vv3 = VVs[t][:].rearrange("p (h e) -> p h e", e=65)
            ps3 = ps[:].rearrange("p (h d) -> p h d", d=64)
            nc.vector.tensor_copy(vv3[:, half * 8:half * 8 + 8, 0:64], ps3[:])

        # ---- phase 1 emission -------------------------------------------
        # Q pass 0 (kk 0-3) for all m as the first qt/wq half arrives;
        # pass 1 for m=0, then K m0 -> attention unit 0 can start.
        for k in range(4):
            for n0 in (0, 512):
                q_chunk(0, n0, k, 1, k == 0)
        for m in range(1, EC):
            q_pass(m, 0)
        q_pass(0, 1)
        k_slot(0)

        # ---- out-projection emitters -------------------------------------
        wo_t = [None]
        yac = []   # bf16 accumulators for m=0..6, allocated in wop at u==4
        yts = []   # two rotating bf16 output staging tiles

        def o_chunk(m, n0, k0, nk):
            ps = psF.tile([128, 512], F32, tag="f", name=f"pso{m}_{n0}_{k0}")
            for k in range(k0, k0 + nk):
                nc.tensor.matmul(
                    ps[:], wo_t[0][:, k, m * 128:(m + 1) * 128],
                    OTs[k][:, n0:n0 + 512],
                    start=(k == k0), stop=(k == k0 + nk - 1))
            if k0 == 0 and nk < EC:
                with nc.allow_low_precision(reason="bf16 out-proj accum"):
                    nc.vector.tensor_copy(yac[m][:, n0:n0 + 512], ps[:])
            elif k0 + nk < EC:
                with nc.allow_low_precision(reason="bf16 out-proj accum"):
                    nc.vector.tensor_add(yac[m][:, n0:n0 + 512],
                                         yac[m][:, n0:n0 + 512], ps[:])
            elif nk == EC:  # m=7: whole contraction in one chunk
                yt = yts[n0 // 512]
                with nc.allow_low_precision(reason="bf16 output"):
                    nc.vector.tensor_scalar_add(
                        yt[:], ps[:], bo_s[:, m:m + 1])
                nc.sync.dma_start(yT[m * 128:(m + 1) * 128, n0:n0 + 512],
                                  yt[:])
            else:
                # finish in place: yac[m] slice becomes the output staging
                dst = yac[m][:, n0:n0 + 512]
                with nc.allow_low_precision(reason="bf16 output"):
                    nc.vector.scalar_tensor_tensor(
                        dst, ps[:], bo_s[:, m:m + 1], dst,
                        op0=mybir.AluOpType.add, op1=mybir.AluOpType.add)
                if m == EC - 2 and n0 == 512:
                    # very last output chunk: split so the final drain
                    # waits on a short 64-col transfer, not 512
                    nc.sync.dma_start(
                        yT[m * 128:(m + 1) * 128, n0:n0 + 448],
                        yac[m][:, n0:n0 + 448])
                    nc.sync.dma_start(
                        yT[m * 128:(m + 1) * 128, n0 + 448:n0 + 512],
                        yac[m][:, n0 + 448:n0 + 512])
                else:
                    nc.sync.dma_start(
                        yT[m * 128:(m + 1) * 128, n0:n0 + 512], dst)

        # ---- attention: units of (head pair j, query half qh) ------------
        def emit_av_kt(st, kt):
            j, ptiles, avs = st["j"], st["pt"], st["avs"]
            for pair in (0, 1):
                av = avs[pair]
                for qi in (0, 1):
                    for hh in (0, 1):
                        idx = kt * 4 + qi * 2 + hh
                        q0 = hh * 512 + (pair * 2 + qi) * 128
                        nc.tensor.matmul(
                            av[:, (qi * 2 + hh) * 65:(qi * 2 + hh + 1) * 65],
                            ptiles[kt][:, q0:q0 + 128],
                            VVs[kt][:, (2 * j + hh) * 65:(2 * j + hh + 1) * 65],
                            start=(idx == 0), stop=(idx == KTn * 4 - 1),
                            skip_group_check=True)

        def finalize_av(st):
            j, qh, avs, oj = st["j"], st["qh"], st["avs"], st["oj"]
            for pair in (0, 1):
                av = avs[pair]
                av3 = av[:].rearrange("p (x c) -> p x c", c=65)
                rc = nrm.tile([128, 4], F32, tag="rc",
                              name=f"rc{j}_{qh}_{pair}")
                nc.vector.reciprocal(
                    rc[:].rearrange("p (a b) -> p a b", b=1), av3[:, :, 64:65])
                for qi in (0, 1):
                    ql = pair * 2 + qi
                    qc = qh * 4 + ql
                    for hh in (0, 1):
                        i = qi * 2 + hh
                        nc.vector.tensor_scalar_mul(
                            oj[:, ql, hh * 64:hh * 64 + 64],
                            av[:, i * 65:i * 65 + 64], rc[:, i:i + 1])
                    nc.sync.dma_start_transpose(
                        OTs[j][:, qc * 128:(qc + 1) * 128], oj[:, ql, :])

        HIPRI = 1 << 20
        units = [(j, qh) for j in range(EC) for qh in (0, 1)]

        # Filler inventory: (emission_deadline_unit, closure). Paced pops
        # drain one item per two kt-steps; any item whose deadline arrives
        # is force-emitted at the top of that unit (emission must precede
        # the first reader -- dependency tracking is program-order
        # directional). List order is psF-execution order: m1 fillers
        # before the V block so attention unit 2 isn't gated on V-proj.
        fillers = [
            (2, lambda: q_pass(1, 1)),
            (2, lambda: k_slot(1)),
        ]
        for t in range(KTn):
            fillers.append((1, lambda t=t: v_chunk(t, 0)))
            fillers.append((1, lambda t=t: v_chunk(t, 1)))
        for m in range(2, EC):
            fillers.append((2 * m, lambda m=m: q_pass(m, 1)))
            fillers.append((2 * m, lambda m=m: k_slot(m)))
        oA = [lambda m=m, n0=n0: o_chunk(m, n0, 0, 4)
              for m in range(EC - 1) for n0 in (0, 512)]
        oB = [lambda m=m, n0=n0: o_chunk(m, n0, 4, 3)
              for m in range(EC - 1) for n0 in (0, 512)]
        oC = [lambda n0=n0: o_chunk(EC - 1, n0, 0, EC) for n0 in (0, 512)]
        oC += [lambda m=m, n0=n0: o_chunk(m, n0, 7, 1)
               for m in range(EC - 1) for n0 in (0, 512)]

        prev = None
        for u, (j, qh) in enumerate(units):
            ptiles = []
            if u == 9:
                fillers.extend((10 ** 9, c) for c in oA)
            if u == 15:
                fillers.extend((10 ** 9, c) for c in oB)
            due = [f for f in fillers if f[0] <= u]
            if due:
                fillers = [f for f in fillers if f[0] > u]
                for _, c in due:
                    c()
            if u == 4:
                # vt/wv no longer needed; reuse the SBUF for wo/yac/yt.
                inpV_ctx.close()
                wo_pool = ctx.enter_context(tc.tile_pool(name="wop", bufs=1))
                wo_t[0] = wo_pool.tile([128, EC, E], BF16, name="wo")
                wov = woT[:].rearrange("(c p) e -> p c e", p=128)
                nc.scalar.dma_start(wo_t[0][:, 0:4, :], wov[:, 0:4, :])
                nc.scalar.dma_start(wo_t[0][:, 4:8, :], wov[:, 4:8, :])
                yac.extend(wo_pool.tile([128, Q], BF16, name=f"yac{m}")
                           for m in range(EC - 1))
                yts.extend(wo_pool.tile([128, 512], BF16, name=f"yt{i}")
                           for i in range(2))
            for kt in range(KTn):
                with tc.high_priority(offset=HIPRI):
                    pe = psE.tile([128, 1024], F32, tag="e",
                                  name=f"pe{j}_{qh}_{kt}")
                    for hh in (0, 1):
                        off = hh * 64
                        nc.tensor.matmul(
                            pe[:, hh * 512:hh * 512 + 512],
                            KTs[j][off:off + 64, kt * 128:(kt + 1) * 128],
                            QTs[j][off:off + 64, qh * 512:qh * 512 + 512])
                    pt = pp.tile([128, 1024], BF16, tag=f"P{qh}_{kt}",
                                 name=f"pt{j}_{qh}_{kt}")
                    nc.scalar.activation(
                        pt[:], pe[:], mybir.ActivationFunctionType.Exp,
                        bias=mb_s[:, kt:kt + 1], scale=0.125)
                    ptiles.append(pt)
                    if prev is not None:
                        if kt == 0:
                            prev["avs"] = [
                                psA.tile([128, 260], F32, tag=f"a{pr}",
                                         name=f"av{prev['j']}_{prev['qh']}_{pr}")
                                for pr in (0, 1)]
                        emit_av_kt(prev, kt)
                if kt % 2 == 1 and fillers:
                    fillers.pop(0)[1]()
            if prev is not None:
                with tc.high_priority(offset=HIPRI):
                    finalize_av(prev)
            prev = dict(j=j, qh=qh, pt=ptiles, avs=None,
                        oj=nrm.tile([128, 4, 128], BF16, tag="oj",
                                    name=f"oj{j}_{qh}"))
        with tc.high_priority(offset=HIPRI):
            prev["avs"] = [psA.tile([128, 260], F32, tag=f"a{pr}",
                                    name=f"av_last_{pr}") for pr in (0, 1)]
            for kt in range(KTn):
                emit_av_kt(prev, kt)
            finalize_av(prev)
        while fillers:
            fillers.pop(0)[1]()
        for c in oC:
            c()

    nc.compile()
    return nc


_PROG_CACHE = {}


def _get_program(Kpad, ntrim):
    key = (Kpad, ntrim)
    if key not in _PROG_CACHE:
        _PROG_CACHE[key] = build_program(Kpad, ntrim)
    return _PROG_CACHE[key]


def prepare_inputs(query, keys, values, mask, Wq, bq, Wk, bk, Wv, bv, Wo, bo):
    """Host-side sharding/layout prep. Returns (Kpad, ntrim, in_maps)."""
    f32 = np.float32
    query = np.asarray(query, f32)
    keys = np.asarray(keys, f32)
    values = np.asarray(values, f32)
    mask = np.asarray(mask)

    idxs = [np.nonzero(mask[b] != 0)[0] for b in range(B)]
    nmax = max(len(i) for i in idxs)
    Kpad = max(256, ((max(nmax, 1) + 127) // 128) * 128)
    KTn = Kpad // 128
    ntrim = min(Kpad, ((max(nmax, 1) + 3) // 4) * 4)

    kTb = np.zeros((B, E, Kpad), BF16NP)
    vTb = np.zeros((B, E, Kpad), BF16NP)
    mbb = np.full((B, Kpad), -1e9, f32)
    for b in range(B):
        n = len(idxs[b])
        kTb[b, :, :n] = keys[b][idxs[b]].T.astype(BF16NP)
        vTb[b, :, :n] = values[b][idxs[b]].T.astype(BF16NP)
        mbb[b, :n] = 0.0
    mb2 = np.ascontiguousarray(mbb.reshape(B, KTn, 128).transpose(0, 2, 1))

    WqT = np.ascontiguousarray(np.asarray(Wq, f32).T.astype(BF16NP))
    WkT = np.ascontiguousarray(np.asarray(Wk, f32).T.astype(BF16NP))
    WvT = np.ascontiguousarray(np.asarray(Wv, f32).T.astype(BF16NP))
    WoT = np.ascontiguousarray(np.asarray(Wo, f32).T.astype(BF16NP))
    bq2 = np.ascontiguousarray(np.asarray(bq, f32).reshape(EC, 128).T)
    bk2 = np.ascontiguousarray(np.asarray(bk, f32).reshape(EC, 128).T)
    # fold V bias through the output projection: y += (Wo @ bv + bo)
    bo_f = np.asarray(bo, f32) + np.asarray(Wo, f32) @ np.asarray(bv, f32)
    bo2 = np.ascontiguousarray(bo_f.reshape(EC, 128).T)

    in_maps = []
    for c in range(NCORES):
        b, h = c // 2, c % 2
        in_maps.append(dict(
            qT=np.ascontiguousarray(
                query[b, h * Q:(h + 1) * Q, :].T.astype(BF16NP)),
            kT=kTb[b], vT=vTb[b], mb=mb2[b],
            wqT=WqT, wkT=WkT, wvT=WvT, woT=WoT,
            bq2=bq2, bk2=bk2, bo2=bo2,
        ))
    return Kpad, ntrim, in_maps


def kernel(query, keys, values, mask, Wq, bq, Wk, bk, Wv, bv, Wo, bo):
    Kpad, ntrim, in_maps = prepare_inputs(query, keys, values, mask,
                                          Wq, bq, Wk, bk, Wv, bv, Wo, bo)
    nc = _get_program(Kpad, ntrim)
    res = run_bass_kernel_spmd(nc, in_maps, list(range(NCORES)))
    out = np.empty((B, S, E), np.float32)
    for c in range(NCORES):
        b, h = c // 2, c % 2
        out[b, h * Q:(h + 1) * Q, :] = \
            res.results[c]["yT"].T.astype(np.float32)
    return out


# revision 31
# speedup vs baseline: 1.0189x; 1.0014x over previous
"""Multi-head attention on 8 TRN2 NeuronCores (Bass/Tile).

Sharding: core c handles batch b = c//2 and query-half h = c%2 (1024 query
tokens), all 16 heads. K/V projections are per-batch and duplicated across
the two cores sharing a batch; no cross-core communication (pair-AllGather
dedup was prototyped but the cost model prices a collective at 15us +
total_bytes/40GB/s, which never pays for the ~15-30us of PE it saves).

Design notes (v9, evolved from v3):
- All matmul operands bf16 (PSUM fp32). Keys compacted on host via the 0/1
  mask; pad keys killed by a -1e9 per-partition bias folded into Exp.
- V-proj bias folded into output bias host-side (bo' = bo + Wo @ bv).
- Energy computed transposed ([key, query] tiles); AV accumulates
  out[q, 65] per head with a ones column carrying the softmax denominator.
- K-projection free dims trimmed to the actual key count (ntrim); KT pad
  columns memset to zero. kt input tile also trimmed.
- Input DMAs split across the two HWDGE queues: SP carries qt/wq/kt/wk +
  transposes + output; the Activation queue carries vt/wv/wo. (The Pool
  SWDGE path was tried and CORRUPTS data under concurrent consumers --
  it races the cross-engine readers; the Act queue is a proper HWDGE
  path and is safe.)
- Out-projection split in three k-chunk groups per (m, n0): A=k0-3 (ready
  once OTs[0..3] final, after unit 9), B=k4-6 (after unit 15), C=k7
  (tail), accumulating into bf16 SBUF tiles (yac). m=7 runs entirely at
  the tail (its yac slot didn't fit in SBUF). Output yT is bf16 (host
  converts to f32); total extra error ~0.2-0.3% rms, well inside 2e-2.
- Attention phase is PE-bound (PE busy ~207us > Act exp stream ~151us).
  Q-proj runs as two 4-kk passes so pass 0 starts when the first qt/wq
  half lands. Phase 1 carries Q m0 per-kk + pass0 all m + pass1 m0 +
  K m0; the rest (Q pass1 m1-7, K m1-7, V t0-8, out-proj groups A/B)
  drain as a deadline-tagged filler deque, one chunk per two kt-steps;
  an item is force-emitted at the top of the unit that first reads its
  output (dependency tracking is program-order directional: a read
  emitted before its writer races it, which shows up as NaN on HW while
  the timeline simulator still passes).
- The PE p-state model punishes sparse matmul streams: a gap resets the
  ramp and subsequent matmuls run at half clock for ~3us. Prefer dense
  chunks (>=4 matmuls per psF tile, PE-bound vs the DVE consumer) over
  per-kk trickles.
"""

import sys

sys.path.insert(0, "/opt/trn_rl_repo")

from contextlib import ExitStack

import ml_dtypes
import numpy as np

import concourse.bass as bass  # noqa: F401
import concourse.tile as tile
from concourse import bacc, mybir
from concourse.bass_utils import run_bass_kernel_spmd

E = 1024          # embed dim
HEADS = 16
HD = 64           # head dim
B = 4
S = 2048
NCORES = 8
Q = (B * S) // NCORES  # query tokens per core
EC = E // 128     # embed chunks of 128
F32 = mybir.dt.float32
BF16 = mybir.dt.bfloat16
BF16NP = ml_dtypes.bfloat16


def build_program(Kpad, ntrim):
    """Per-core Bass program (identical on all 8 cores)."""
    KTn = Kpad // 128
    nc = bacc.Bacc("TRN2", target_bir_lowering=False, debug=False,
                   num_devices=NCORES, dynamic_dma_scratch_size=2048)

    qT = nc.dram_tensor("qT", [E, Q], BF16, kind="ExternalInput").ap()
    kT = nc.dram_tensor("kT", [E, Kpad], BF16, kind="ExternalInput").ap()
    vT = nc.dram_tensor("vT", [E, Kpad], BF16, kind="ExternalInput").ap()
    wqT = nc.dram_tensor("wqT", [E, E], BF16, kind="ExternalInput").ap()
    wkT = nc.dram_tensor("wkT", [E, E], BF16, kind="ExternalInput").ap()
    wvT = nc.dram_tensor("wvT", [E, E], BF16, kind="ExternalInput").ap()
    woT = nc.dram_tensor("woT", [E, E], BF16, kind="ExternalInput").ap()
    bq2 = nc.dram_tensor("bq2", [128, EC], F32, kind="ExternalInput").ap()
    bk2 = nc.dram_tensor("bk2", [128, EC], F32, kind="ExternalInput").ap()
    bo2 = nc.dram_tensor("bo2", [128, EC], F32, kind="ExternalInput").ap()
    mb = nc.dram_tensor("mb", [128, KTn], F32, kind="ExternalInput").ap()
    yT = nc.dram_tensor("yT", [E, Q], BF16, kind="ExternalOutput").ap()

    with tile.TileContext(nc) as tc, ExitStack() as ctx:
        sml = ctx.enter_context(tc.tile_pool(name="sml", bufs=1))
        big = ctx.enter_context(tc.tile_pool(name="big", bufs=1))

        # ---- PSUM pools: psE 2x[128,1024]=4 banks, psA 2x1=2, psF 2x1=2
        psE = ctx.enter_context(tc.tile_pool(name="psE", bufs=2, space="PSUM"))
        psA = ctx.enter_context(tc.tile_pool(name="psA", bufs=1, space="PSUM"))
        psF = ctx.enter_context(tc.tile_pool(name="psF", bufs=2, space="PSUM"))

        inp = ctx.enter_context(tc.tile_pool(name="inp", bufs=1))
        pp = ctx.enter_context(tc.tile_pool(name="pp", bufs=1))
        nrm = ctx.enter_context(tc.tile_pool(name="nrm", bufs=2))
        # vt/wv free mid-attention; inpV sits atop the pool stack so its
        # SBUF can be reused for wo/yac/yt.
        inpV_ctx = ExitStack()
        inpV = inpV_ctx.enter_context(tc.tile_pool(name="inpV", bufs=1))

        # ---- big input DMAs (single SP/HWDGE queue, consumption order) ---
        qt_t, wq_t = [], []
        qv = qT[:].rearrange("(c p) q -> p c q", p=128)
        wqv = wqT[:].rearrange("(c p) e -> p c e", p=128)
        bq_s = sml.tile([128, EC], F32, name="bq_s")
        bk_s = sml.tile([128, EC], F32, name="bk_s")
        bo_s = sml.tile([128, EC], F32, name="bo_s")
        mb_s = sml.tile([128, KTn], F32, name="mb_s")
        for h in (0, 1):
            t = inp.tile([128, 4, Q], BF16, name=f"qt{h}")
            w = inp.tile([128, 4, E], BF16, name=f"wq{h}")
            for kk in range(4):
                nc.sync.dma_start(t[:, kk, :], qv[:, 4 * h + kk, :])
                nc.sync.dma_start(w[:, kk, :], wqv[:, 4 * h + kk, :])
                if h == 0 and kk == 1:
                    nc.sync.dma_start(bq_s[:], bq2[:])
                    nc.sync.dma_start(bk_s[:], bk2[:])
                    nc.sync.dma_start(bo_s[:], bo2[:])
                    nc.sync.dma_start(mb_s[:], mb[:])
            qt_t.append(t)
            wq_t.append(w)
        kt_t = inp.tile([128, EC, ntrim], BF16, name="kt")
        kv4 = kT[:].rearrange("(c p) k -> p c k", p=128)
        nc.sync.dma_start(kt_t[:, 0:4, :], kv4[:, 0:4, 0:ntrim])
        nc.sync.dma_start(kt_t[:, 4:8, :], kv4[:, 4:8, 0:ntrim])
        wk_t = inp.tile([128, EC, E], BF16, name="wk")
        wkv = wkT[:].rearrange("(c p) e -> p c e", p=128)
        nc.sync.dma_start(wk_t[:, 0:4, :], wkv[:, 0:4, :])
        nc.sync.dma_start(wk_t[:, 4:8, :], wkv[:, 4:8, :])
        vt_t = inpV.tile([128, EC, Kpad], BF16, name="vt")
        vv4 = vT[:].rearrange("(c p) k -> p c k", p=128)
        wv_t = inpV.tile([128, EC, E], BF16, name="wv")
        wvv = wvT[:].rearrange("(c p) e -> p c e", p=128)
        for c0 in (0, 2, 4, 6):
            nc.scalar.dma_start(vt_t[:, c0:c0 + 2, :], vv4[:, c0:c0 + 2, :])
            nc.scalar.dma_start(wv_t[:, c0:c0 + 2, :], wvv[:, c0:c0 + 2, :])

        # ---- persistent SBUF tensors -------------------------------------
        QTs = [big.tile([128, Q], BF16, name=f"QT{m}") for m in range(EC)]
        KTs = [big.tile([128, Kpad], BF16, name=f"KT{m}") for m in range(EC)]
        VVs = [big.tile([128, HEADS * 65], BF16, name=f"VV{t}")
               for t in range(KTn)]
        OTs = [big.tile([128, Q], BF16, name=f"OT{m}") for m in range(EC)]
        # ones columns of the V tiles, written once before anything reads VV
        for t in range(KTn):
            vv3 = VVs[t][:].rearrange("p (h e) -> p h e", e=65)
            nc.vector.memset(vv3[:, :, 64:65], 1.0)

        # K-proj free-dim chunks, trimmed to ntrim
        kchunks = []
        for n0 in range(0, ntrim, 512):
            kchunks.append((n0, min(512, ntrim - n0)))

        # ---- projection emitters -----------------------------------------
        def q_pass(m, half):
            """Half-contraction Q-proj pass: kk in [half*4, half*4+4).
            Pass 0 writes QTs[m] (with bias); pass 1 accumulates."""
            for n0 in (0, 512):
                q_chunk(m, n0, half * 4, 4, half == 0)

        def q_chunk(m, n0, k0=0, nk=EC, first=True):
            ps = psF.tile([128, 512], F32, tag="f", name=f"psq{m}_{n0}_{k0}")
            for kk in range(k0, k0 + nk):
                nc.tensor.matmul(
                    ps[:], wq_t[kk // 4][:, kk % 4, m * 128:(m + 1) * 128],
                    qt_t[kk // 4][:, kk % 4, n0:n0 + 512],
                    start=(kk == k0), stop=(kk == k0 + nk - 1))
            if first:
                nc.vector.tensor_scalar_add(
                    QTs[m][:, n0:n0 + 512], ps[:], bq_s[:, m:m + 1])
            else:
                with nc.allow_low_precision(reason="bf16 proj accum"):
                    nc.vector.tensor_add(
                        QTs[m][:, n0:n0 + 512], QTs[m][:, n0:n0 + 512], ps[:])

        def k_chunk(m, n0, nn, last):
            ps = psF.tile([128, 512], F32, tag="f", name=f"psk{m}_{n0}")
            for kk in range(EC):
                nc.tensor.matmul(
                    ps[:, 0:nn], wk_t[:, kk, m * 128:(m + 1) * 128],
                    kt_t[:, kk, n0:n0 + nn],
                    start=(kk == 0), stop=(kk == EC - 1))
            nc.vector.tensor_scalar_add(
                KTs[m][:, n0:n0 + nn], ps[:, 0:nn], bk_s[:, m:m + 1])
            if last and ntrim < Kpad:
                nc.vector.memset(KTs[m][:, ntrim:Kpad], 0.0)

        def k_slot(m):
            for ci, (n0, nn) in enumerate(kchunks):
                k_chunk(m, n0, nn, ci == len(kchunks) - 1)

        def v_chunk(t, half):
            """V-proj chunk: heads half*8..half*8+8 of key tile t."""
            n0 = half * 512
            ps = psF.tile([128, 512], F32, tag="f", name=f"psv{t}_{half}")
            for kk in range(EC):
                nc.tensor.matmul(
                    ps[:], vt_t[:, kk, t * 128:(t + 1) * 128],
                    wv_t[:, kk, n0:n0 + 512],
                    start=(kk == 0), stop=(kk == EC - 1))
            vv3 = VVs[t][:].rearrange("p (h e) -> p h e", e=65)
            ps3 = ps[:].rearrange("p (h d) -> p h d", d=64)
            nc.vector.tensor_copy(vv3[:, half * 8:half * 8 + 8, 0:64], ps3[:])

        # ---- phase 1 emission -------------------------------------------
        # Q pass 0 (kk 0-3) for all m as the first qt/wq half arrives;
        # pass 1 for m=0, then K m0 -> attention unit 0 can start.
        for k in range(4):
            for n0 in (0, 512):
                q_chunk(0, n0, k, 1, k == 0)
        for m in range(1, EC):
            q_pass(m, 0)
        q_pass(0, 1)
        k_slot(0)

        # ---- out-projection emitters -------------------------------------
        wo_t = [None]
        yac = []   # bf16 accumulators for m=0..6, allocated in wop at u==4
        yts = []   # two rotating bf16 output staging tiles

        def o_chunk(m, n0, k0, nk):
            ps = psF.tile([128, 512], F32, tag="f", name=f"pso{m}_{n0}_{k0}")
            for k in range(k0, k0 + nk):
                nc.tensor.matmul(
                    ps[:], wo_t[0][:, k, m * 128:(m + 1) * 128],
                    OTs[k][:, n0:n0 + 512],
                    start=(k == k0), stop=(k == k0 + nk - 1))
            if k0 == 0 and nk < EC:
                with nc.allow_low_precision(reason="bf16 out-proj accum"):
                    nc.vector.tensor_copy(yac[m][:, n0:n0 + 512], ps[:])
            elif k0 + nk < EC:
                with nc.allow_low_precision(reason="bf16 out-proj accum"):
                    nc.vector.tensor_add(yac[m][:, n0:n0 + 512],
                                         yac[m][:, n0:n0 + 512], ps[:])
            elif nk == EC:  # m=7: whole contraction in one chunk
                yt = yts[n0 // 512]
                with nc.allow_low_precision(reason="bf16 output"):
                    nc.vector.tensor_scalar_add(
                        yt[:], ps[:], bo_s[:, m:m + 1])
                if n0 == 512:
                    # very last output chunk: split so the final drain
                    # waits on a short 64-col transfer, not 512
                    nc.sync.dma_start(
                        yT[m * 128:(m + 1) * 128, n0:n0 + 448],
                        yt[:, 0:448])
                    nc.sync.dma_start(
                        yT[m * 128:(m + 1) * 128, n0 + 448:n0 + 512],
                        yt[:, 448:512])
                else:
                    nc.sync.dma_start(
                        yT[m * 128:(m + 1) * 128, n0:n0 + 512], yt[:])
            else:
                # finish in place: yac[m] slice becomes the output staging
                dst = yac[m][:, n0:n0 + 512]
                with nc.allow_low_precision(reason="bf16 output"):
                    nc.vector.scalar_tensor_tensor(
                        dst, ps[:], bo_s[:, m:m + 1], dst,
                        op0=mybir.AluOpType.add, op1=mybir.AluOpType.add)
                nc.sync.dma_start(yT[m * 128:(m + 1) * 128, n0:n0 + 512],
                                  dst)

        # ---- attention: units of (head pair j, query half qh) ------------
        def emit_av_kt(st, kt):
            j, ptiles, avs = st["j"], st["pt"], st["avs"]
            for pair in (0, 1):
                av = avs[pair]
                for qi in (0, 1):
                    for hh in (0, 1):
                        idx = kt * 4 + qi * 2 + hh
                        q0 = hh * 512 + (pair * 2 + qi) * 128
                        nc.tensor.matmul(
                            av[:, (qi * 2 + hh) * 65:(qi * 2 + hh + 1) * 65],
                            ptiles[kt][:, q0:q0 + 128],
                            VVs[kt][:, (2 * j + hh) * 65:(2 * j + hh + 1) * 65],
                            start=(idx == 0), stop=(idx == KTn * 4 - 1),
                            skip_group_check=True)

        def finalize_av(st):
            j, qh, avs, oj = st["j"], st["qh"], st["avs"], st["oj"]
            for pair in (0, 1):
                av = avs[pair]
                av3 = av[:].rearrange("p (x c) -> p x c", c=65)
                rc = nrm.tile([128, 4], F32, tag="rc",
                              name=f"rc{j}_{qh}_{pair}")
                nc.vector.reciprocal(
                    rc[:].rearrange("p (a b) -> p a b", b=1), av3[:, :, 64:65])
                for qi in (0, 1):
                    ql = pair * 2 + qi
                    qc = qh * 4 + ql
                    for hh in (0, 1):
                        i = qi * 2 + hh
                        nc.vector.tensor_scalar_mul(
                            oj[:, ql, hh * 64:hh * 64 + 64],
                            av[:, i * 65:i * 65 + 64], rc[:, i:i + 1])
                    nc.sync.dma_start_transpose(
                        OTs[j][:, qc * 128:(qc + 1) * 128], oj[:, ql, :])

        HIPRI = 1 << 20
        units = [(j, qh) for j in range(EC) for qh in (0, 1)]

        # Filler inventory: (emission_deadline_unit, closure). Paced pops
        # drain one item per two kt-steps; any item whose deadline arrives
        # is force-emitted at the top of that unit (emission must precede
        # the first reader -- dependency tracking is program-order
        # directional). List order is psF-execution order: m1 fillers
        # before the V block so attention unit 2 isn't gated on V-proj.
        fillers = [
            (2, lambda: q_pass(1, 1)),
            (2, lambda: k_slot(1)),
        ]
        for t in range(KTn):
            fillers.append((1, lambda t=t: v_chunk(t, 0)))
            fillers.append((1, lambda t=t: v_chunk(t, 1)))
        for m in range(2, EC):
            fillers.append((2 * m, lambda m=m: q_pass(m, 1)))
            fillers.append((2 * m, lambda m=m: k_slot(m)))
        oA = [lambda m=m, n0=n0: o_chunk(m, n0, 0, 4)
              for m in range(EC - 1) for n0 in (0, 512)]
        oB = [lambda m=m, n0=n0: o_chunk(m, n0, 4, 3)
              for m in range(EC - 1) for n0 in (0, 512)]
        oC = [lambda m=m, n0=n0: o_chunk(m, n0, 7, 1)
              for m in range(EC - 1) for n0 in (0, 512)]
        oC += [lambda n0=n0: o_chunk(EC - 1, n0, 0, EC) for n0 in (0, 512)]

        prev = None
        for u, (j, qh) in enumerate(units):
            ptiles = []
            if u == 9:
                fillers.extend((10 ** 9, c) for c in oA)
            if u == 15:
                fillers.extend((10 ** 9, c) for c in oB)
            due = [f for f in fillers if f[0] <= u]
            if due:
                fillers = [f for f in fillers if f[0] > u]
                for _, c in due:
                    c()
            if u == 4:
                # vt/wv no longer needed; reuse the SBUF for wo/yac/yt.
                inpV_ctx.close()
                wo_pool = ctx.enter_context(tc.tile_pool(name="wop", bufs=1))
                wo_t[0] = wo_pool.tile([128, EC, E], BF16, name="wo")
                wov = woT[:].rearrange("(c p) e -> p c e", p=128)
                nc.scalar.dma_start(wo_t[0][:, 0:4, :], wov[:, 0:4, :])
                nc.scalar.dma_start(wo_t[0][:, 4:8, :], wov[:, 4:8, :])
                yac.extend(wo_pool.tile([128, Q], BF16, name=f"yac{m}")
                           for m in range(EC - 1))
                yts.extend(wo_pool.tile([128, 512], BF16, name=f"yt{i}")
                           for i in range(2))
            for kt in range(KTn):
                with tc.high_priority(offset=HIPRI):
                    pe = psE.tile([128, 1024], F32, tag="e",
                                  name=f"pe{j}_{qh}_{kt}")
                    for hh in (0, 1):
                        off = hh * 64
                        nc.tensor.matmul(
                            pe[:, hh * 512:hh * 512 + 512],
                            KTs[j][off:off + 64, kt * 128:(kt + 1) * 128],
                            QTs[j][off:off + 64, qh * 512:qh * 512 + 512])
                    pt = pp.tile([128, 1024], BF16, tag=f"P{qh}_{kt}",
                                 name=f"pt{j}_{qh}_{kt}")
                    nc.scalar.activation(
                        pt[:], pe[:], mybir.ActivationFunctionType.Exp,
                        bias=mb_s[:, kt:kt + 1], scale=0.125)
                    ptiles.append(pt)
                    if prev is not None:
                        if kt == 0:
                            prev["avs"] = [
                                psA.tile([128, 260], F32, tag=f"a{pr}",
                                         name=f"av{prev['j']}_{prev['qh']}_{pr}")
                                for pr in (0, 1)]
                        emit_av_kt(prev, kt)
                if kt % 2 == 1 and fillers:
                    fillers.pop(0)[1]()
            if prev is not None:
                with tc.high_priority(offset=HIPRI):
                    finalize_av(prev)
            prev = dict(j=j, qh=qh, pt=ptiles, avs=None,
                        oj=nrm.tile([128, 4, 128], BF16, tag="oj",
                                    name=f"oj{j}_{qh}"))
        with tc.high_priority(offset=HIPRI):
            prev["avs"] = [psA.tile([128, 260], F32, tag=f"a{pr}",
                                    name=f"av_last_{pr}") for pr in (0, 1)]
            for kt in range(KTn):
                emit_av_kt(prev, kt)
            finalize_av(prev)
        while fillers:
            fillers.pop(0)[1]()
        for c in oC:
            c()

    nc.compile()
    return nc


_PROG_CACHE = {}


def _get_program(Kpad, ntrim):
    key = (Kpad, ntrim)
    if key not in _PROG_CACHE:
        _PROG_CACHE[key] = build_program(Kpad, ntrim)
    return _PROG_CACHE[key]


def prepare_inputs(query, keys, values, mask, Wq, bq, Wk, bk, Wv, bv, Wo, bo):
    """Host-side sharding/layout prep. Returns (Kpad, ntrim, in_maps)."""
    f32 = np.float32
    query = np.asarray(query, f32)
    keys = np.asarray(keys, f32)
    values = np.asarray(values, f32)
    mask = np.asarray(mask)

    idxs = [np.nonzero(mask[b] != 0)[0] for b in range(B)]
    nmax = max(len(i) for i in idxs)
    Kpad = max(256, ((max(nmax, 1) + 127) // 128) * 128)
    KTn = Kpad // 128
    ntrim = min(Kpad, ((max(nmax, 1) + 3) // 4) * 4)

    kTb = np.zeros((B, E, Kpad), BF16NP)
    vTb = np.zeros((B, E, Kpad), BF16NP)
    mbb = np.full((B, Kpad), -1e9, f32)
    for b in range(B):
        n = len(idxs[b])
        kTb[b, :, :n] = keys[b][idxs[b]].T.astype(BF16NP)
        vTb[b, :, :n] = values[b][idxs[b]].T.astype(BF16NP)
        mbb[b, :n] = 0.0
    mb2 = np.ascontiguousarray(mbb.reshape(B, KTn, 128).transpose(0, 2, 1))

    WqT = np.ascontiguousarray(np.asarray(Wq, f32).T.astype(BF16NP))
    WkT = np.ascontiguousarray(np.asarray(Wk, f32).T.astype(BF16NP))
    WvT = np.ascontiguousarray(np.asarray(Wv, f32).T.astype(BF16NP))
    WoT = np.ascontiguousarray(np.asarray(Wo, f32).T.astype(BF16NP))
    bq2 = np.ascontiguousarray(np.asarray(bq, f32).reshape(EC, 128).T)
    bk2 = np.ascontiguousarray(np.asarray(bk, f32).reshape(EC, 128).T)
    # fold V bias through the output projection: y += (Wo @ bv + bo)
    bo_f = np.asarray(bo, f32) + np.asarray(Wo, f32) @ np.asarray(bv, f32)
    bo2 = np.ascontiguousarray(bo_f.reshape(EC, 128).T)

    in_maps = []
    for c in range(NCORES):
        b, h = c // 2, c % 2
        in_maps.append(dict(
            qT=np.ascontiguousarray(
                query[b, h * Q:(h + 1) * Q, :].T.astype(BF16NP)),
            kT=kTb[b], vT=vTb[b], mb=mb2[b],
            wqT=WqT, wkT=WkT, wvT=WvT, woT=WoT,
            bq2=bq2, bk2=bk2, bo2=bo2,
        ))
    return Kpad, ntrim, in_maps


def kernel(query, keys, values, mask, Wq, bq, Wk, bk, Wv, bv, Wo, bo):
    Kpad, ntrim, in_maps = prepare_inputs(query, keys, values, mask,
                                          Wq, bq, Wk, bk, Wv, bv, Wo, bo)
    nc = _get_program(Kpad, ntrim)
    res = run_bass_kernel_spmd(nc, in_maps, list(range(NCORES)))
    out = np.empty((B, S, E), np.float32)
    for c in range(NCORES):
        b, h = c // 2, c % 2
        out[b, h * Q:(h + 1) * Q, :] = \
            res.results[c]["yT"].T.astype(np.float32)
    return out


# revision 32
# speedup vs baseline: 1.0207x; 1.0017x over previous
"""Multi-head attention on 8 TRN2 NeuronCores (Bass/Tile).

Sharding: core c handles batch b = c//2 and query-half h = c%2 (1024 query
tokens), all 16 heads. K/V projections are per-batch and duplicated across
the two cores sharing a batch; no cross-core communication (pair-AllGather
dedup was prototyped but the cost model prices a collective at 15us +
total_bytes/40GB/s, which never pays for the ~15-30us of PE it saves).

Design notes (v9, evolved from v3):
- All matmul operands bf16 (PSUM fp32). Keys compacted on host via the 0/1
  mask; pad keys killed by a -1e9 per-partition bias folded into Exp.
- V-proj bias folded into output bias host-side (bo' = bo + Wo @ bv).
- Energy computed transposed ([key, query] tiles); AV accumulates
  out[q, 65] per head with a ones column carrying the softmax denominator.
- K-projection free dims trimmed to the actual key count (ntrim); KT pad
  columns memset to zero. kt input tile also trimmed.
- Input DMAs split across the two HWDGE queues: SP carries qt/wq/kt/wk +
  transposes + output; the Activation queue carries vt/wv/wo. (The Pool
  SWDGE path was tried and CORRUPTS data under concurrent consumers --
  it races the cross-engine readers; the Act queue is a proper HWDGE
  path and is safe.)
- Out-projection split in three k-chunk groups per (m, n0): A=k0-3 (ready
  once OTs[0..3] final, after unit 9), B=k4-6 (after unit 15), C=k7
  (tail), accumulating into bf16 SBUF tiles (yac). m=7 runs entirely at
  the tail (its yac slot didn't fit in SBUF). Output yT is bf16 (host
  converts to f32); total extra error ~0.2-0.3% rms, well inside 2e-2.
- Attention phase is PE-bound (PE busy ~207us > Act exp stream ~151us).
  Q-proj runs as two 4-kk passes so pass 0 starts when the first qt/wq
  half lands. Phase 1 carries Q m0 per-kk + pass0 all m + pass1 m0 +
  K m0; the rest (Q pass1 m1-7, K m1-7, V t0-8, out-proj groups A/B)
  drain as a deadline-tagged filler deque, one chunk per two kt-steps;
  an item is force-emitted at the top of the unit that first reads its
  output (dependency tracking is program-order directional: a read
  emitted before its writer races it, which shows up as NaN on HW while
  the timeline simulator still passes).
- The PE p-state model punishes sparse matmul streams: a gap resets the
  ramp and subsequent matmuls run at half clock for ~3us. Prefer dense
  chunks (>=4 matmuls per psF tile, PE-bound vs the DVE consumer) over
  per-kk trickles.
"""

import sys

sys.path.insert(0, "/opt/trn_rl_repo")

from contextlib import ExitStack

import ml_dtypes
import numpy as np

import concourse.bass as bass  # noqa: F401
import concourse.tile as tile
from concourse import bacc, mybir
from concourse.bass_utils import run_bass_kernel_spmd

E = 1024          # embed dim
HEADS = 16
HD = 64           # head dim
B = 4
S = 2048
NCORES = 8
Q = (B * S) // NCORES  # query tokens per core
EC = E // 128     # embed chunks of 128
F32 = mybir.dt.float32
BF16 = mybir.dt.bfloat16
BF16NP = ml_dtypes.bfloat16


def build_program(Kpad, ntrim):
    """Per-core Bass program (identical on all 8 cores)."""
    KTn = Kpad // 128
    nc = bacc.Bacc("TRN2", target_bir_lowering=False, debug=False,
                   num_devices=NCORES, dynamic_dma_scratch_size=2048)

    qT = nc.dram_tensor("qT", [E, Q], BF16, kind="ExternalInput").ap()
    kT = nc.dram_tensor("kT", [E, Kpad], BF16, kind="ExternalInput").ap()
    vT = nc.dram_tensor("vT", [E, Kpad], BF16, kind="ExternalInput").ap()
    wqT = nc.dram_tensor("wqT", [E, E], BF16, kind="ExternalInput").ap()
    wkT = nc.dram_tensor("wkT", [E, E], BF16, kind="ExternalInput").ap()
    wvT = nc.dram_tensor("wvT", [E, E], BF16, kind="ExternalInput").ap()
    woT = nc.dram_tensor("woT", [E, E], BF16, kind="ExternalInput").ap()
    bq2 = nc.dram_tensor("bq2", [128, EC], F32, kind="ExternalInput").ap()
    bk2 = nc.dram_tensor("bk2", [128, EC], F32, kind="ExternalInput").ap()
    bo2 = nc.dram_tensor("bo2", [128, EC], F32, kind="ExternalInput").ap()
    mb = nc.dram_tensor("mb", [128, KTn], F32, kind="ExternalInput").ap()
    yT = nc.dram_tensor("yT", [E, Q], BF16, kind="ExternalOutput").ap()

    with tile.TileContext(nc) as tc, ExitStack() as ctx:
        sml = ctx.enter_context(tc.tile_pool(name="sml", bufs=1))
        big = ctx.enter_context(tc.tile_pool(name="big", bufs=1))

        # ---- PSUM pools: psE 2x[128,1024]=4 banks, psA 2x1=2, psF 2x1=2
        psE = ctx.enter_context(tc.tile_pool(name="psE", bufs=2, space="PSUM"))
        psA = ctx.enter_context(tc.tile_pool(name="psA", bufs=1, space="PSUM"))
        psF = ctx.enter_context(tc.tile_pool(name="psF", bufs=2, space="PSUM"))

        inp = ctx.enter_context(tc.tile_pool(name="inp", bufs=1))
        pp = ctx.enter_context(tc.tile_pool(name="pp", bufs=1))
        nrm = ctx.enter_context(tc.tile_pool(name="nrm", bufs=2))
        # vt/wv free mid-attention; inpV sits atop the pool stack so its
        # SBUF can be reused for wo/yac/yt.
        inpV_ctx = ExitStack()
        inpV = inpV_ctx.enter_context(tc.tile_pool(name="inpV", bufs=1))

        # ---- big input DMAs (single SP/HWDGE queue, consumption order) ---
        qt_t, wq_t = [], []
        qv = qT[:].rearrange("(c p) q -> p c q", p=128)
        wqv = wqT[:].rearrange("(c p) e -> p c e", p=128)
        bq_s = sml.tile([128, EC], F32, name="bq_s")
        bk_s = sml.tile([128, EC], F32, name="bk_s")
        bo_s = sml.tile([128, EC], F32, name="bo_s")
        mb_s = sml.tile([128, KTn], F32, name="mb_s")
        for h in (0, 1):
            t = inp.tile([128, 4, Q], BF16, name=f"qt{h}")
            w = inp.tile([128, 4, E], BF16, name=f"wq{h}")
            for kk in range(4):
                nc.sync.dma_start(t[:, kk, :], qv[:, 4 * h + kk, :])
                nc.sync.dma_start(w[:, kk, :], wqv[:, 4 * h + kk, :])
                if h == 0 and kk == 1:
                    nc.sync.dma_start(bq_s[:], bq2[:])
                    nc.sync.dma_start(bk_s[:], bk2[:])
                    nc.sync.dma_start(bo_s[:], bo2[:])
                    nc.sync.dma_start(mb_s[:], mb[:])
            qt_t.append(t)
            wq_t.append(w)
        kt_t = inp.tile([128, EC, ntrim], BF16, name="kt")
        kv4 = kT[:].rearrange("(c p) k -> p c k", p=128)
        nc.sync.dma_start(kt_t[:, 0:4, :], kv4[:, 0:4, 0:ntrim])
        nc.sync.dma_start(kt_t[:, 4:8, :], kv4[:, 4:8, 0:ntrim])
        wk_t = inp.tile([128, EC, E], BF16, name="wk")
        wkv = wkT[:].rearrange("(c p) e -> p c e", p=128)
        nc.sync.dma_start(wk_t[:, 0:4, :], wkv[:, 0:4, :])
        nc.sync.dma_start(wk_t[:, 4:8, :], wkv[:, 4:8, :])
        vt_t = inpV.tile([128, EC, Kpad], BF16, name="vt")
        vv4 = vT[:].rearrange("(c p) k -> p c k", p=128)
        wv_t = inpV.tile([128, EC, E], BF16, name="wv")
        wvv = wvT[:].rearrange("(c p) e -> p c e", p=128)
        for c0 in (0, 2, 4, 6):
            nc.scalar.dma_start(vt_t[:, c0:c0 + 2, :], vv4[:, c0:c0 + 2, :])
            nc.scalar.dma_start(wv_t[:, c0:c0 + 2, :], wvv[:, c0:c0 + 2, :])

        # ---- persistent SBUF tensors -------------------------------------
        QTs = [big.tile([128, Q], BF16, name=f"QT{m}") for m in range(EC)]
        KTs = [big.tile([128, Kpad], BF16, name=f"KT{m}") for m in range(EC)]
        VVs = [big.tile([128, HEADS * 65], BF16, name=f"VV{t}")
               for t in range(KTn)]
        OTs = [big.tile([128, Q], BF16, name=f"OT{m}") for m in range(EC)]
        # ones columns of the V tiles, written once before anything reads VV
        for t in range(KTn):
            vv3 = VVs[t][:].rearrange("p (h e) -> p h e", e=65)
            nc.vector.memset(vv3[:, :, 64:65], 1.0)

        # K-proj free-dim chunks, trimmed to ntrim
        kchunks = []
        for n0 in range(0, ntrim, 512):
            kchunks.append((n0, min(512, ntrim - n0)))

        # ---- projection emitters -----------------------------------------
        def q_pass(m, half):
            """Half-contraction Q-proj pass: kk in [half*4, half*4+4).
            Pass 0 writes QTs[m] (with bias); pass 1 accumulates."""
            for n0 in (0, 512):
                q_chunk(m, n0, half * 4, 4, half == 0)

        def q_chunk(m, n0, k0=0, nk=EC, first=True):
            ps = psF.tile([128, 512], F32, tag="f", name=f"psq{m}_{n0}_{k0}")
            for kk in range(k0, k0 + nk):
                nc.tensor.matmul(
                    ps[:], wq_t[kk // 4][:, kk % 4, m * 128:(m + 1) * 128],
                    qt_t[kk // 4][:, kk % 4, n0:n0 + 512],
                    start=(kk == k0), stop=(kk == k0 + nk - 1))
            if first:
                nc.vector.tensor_scalar_add(
                    QTs[m][:, n0:n0 + 512], ps[:], bq_s[:, m:m + 1])
            else:
                with nc.allow_low_precision(reason="bf16 proj accum"):
                    nc.vector.tensor_add(
                        QTs[m][:, n0:n0 + 512], QTs[m][:, n0:n0 + 512], ps[:])

        def k_chunk(m, n0, nn, last):
            ps = psF.tile([128, 512], F32, tag="f", name=f"psk{m}_{n0}")
            for kk in range(EC):
                nc.tensor.matmul(
                    ps[:, 0:nn], wk_t[:, kk, m * 128:(m + 1) * 128],
                    kt_t[:, kk, n0:n0 + nn],
                    start=(kk == 0), stop=(kk == EC - 1))
            nc.vector.tensor_scalar_add(
                KTs[m][:, n0:n0 + nn], ps[:, 0:nn], bk_s[:, m:m + 1])
            if last and ntrim < Kpad:
                nc.vector.memset(KTs[m][:, ntrim:Kpad], 0.0)

        def k_slot(m):
            for ci, (n0, nn) in enumerate(kchunks):
                k_chunk(m, n0, nn, ci == len(kchunks) - 1)

        def v_chunk(t, half):
            """V-proj chunk: heads half*8..half*8+8 of key tile t."""
            n0 = half * 512
            ps = psF.tile([128, 512], F32, tag="f", name=f"psv{t}_{half}")
            for kk in range(EC):
                nc.tensor.matmul(
                    ps[:], vt_t[:, kk, t * 128:(t + 1) * 128],
                    wv_t[:, kk, n0:n0 + 512],
                    start=(kk == 0), stop=(kk == EC - 1))
            vv3 = VVs[t][:].rearrange("p (h e) -> p h e", e=65)
            ps3 = ps[:].rearrange("p (h d) -> p h d", d=64)
            nc.vector.tensor_copy(vv3[:, half * 8:half * 8 + 8, 0:64], ps3[:])

        # ---- phase 1 emission -------------------------------------------
        # Q pass 0 (kk 0-3) for all m as the first qt/wq half arrives;
        # pass 1 for m=0, then K m0 -> attention unit 0 can start.
        for k in range(4):
            for n0 in (0, 512):
                q_chunk(0, n0, k, 1, k == 0)
        for m in range(1, EC):
            q_pass(m, 0)
        q_pass(0, 1)
        k_slot(0)

        # ---- out-projection emitters -------------------------------------
        wo_t = [None]
        yac = []   # bf16 accumulators for m=0..6, allocated in wop at u==4
        yts = []   # two rotating bf16 output staging tiles

        def o_chunk(m, n0, k0, nk):
            ps = psF.tile([128, 512], F32, tag="f", name=f"pso{m}_{n0}_{k0}")
            for k in range(k0, k0 + nk):
                nc.tensor.matmul(
                    ps[:], wo_t[0][:, k, m * 128:(m + 1) * 128],
                    OTs[k][:, n0:n0 + 512],
                    start=(k == k0), stop=(k == k0 + nk - 1))
            if k0 == 0 and nk < EC:
                with nc.allow_low_precision(reason="bf16 out-proj accum"):
                    nc.vector.tensor_copy(yac[m][:, n0:n0 + 512], ps[:])
            elif k0 + nk < EC:
                with nc.allow_low_precision(reason="bf16 out-proj accum"):
                    nc.vector.tensor_add(yac[m][:, n0:n0 + 512],
                                         yac[m][:, n0:n0 + 512], ps[:])
            elif nk == EC:  # m=7: whole contraction in one chunk
                yt = yts[n0 // 512]
                with nc.allow_low_precision(reason="bf16 output"):
                    nc.vector.tensor_scalar_add(
                        yt[:], ps[:], bo_s[:, m:m + 1])
                nc.sync.dma_start(yT[m * 128:(m + 1) * 128, n0:n0 + 512],
                                  yt[:])
            else:
                # finish in place: yac[m] slice becomes the output staging
                dst = yac[m][:, n0:n0 + 512]
                with nc.allow_low_precision(reason="bf16 output"):
                    nc.vector.scalar_tensor_tensor(
                        dst, ps[:], bo_s[:, m:m + 1], dst,
                        op0=mybir.AluOpType.add, op1=mybir.AluOpType.add)
                nc.sync.dma_start(yT[m * 128:(m + 1) * 128, n0:n0 + 512],
                                  dst)

        # ---- attention: units of (head pair j, query half qh) ------------
        def emit_av_kt(st, kt):
            j, ptiles, avs = st["j"], st["pt"], st["avs"]
            for pair in (0, 1):
                av = avs[pair]
                for qi in (0, 1):
                    for hh in (0, 1):
                        idx = kt * 4 + qi * 2 + hh
                        q0 = hh * 512 + (pair * 2 + qi) * 128
                        nc.tensor.matmul(
                            av[:, (qi * 2 + hh) * 65:(qi * 2 + hh + 1) * 65],
                            ptiles[kt][:, q0:q0 + 128],
                            VVs[kt][:, (2 * j + hh) * 65:(2 * j + hh + 1) * 65],
                            start=(idx == 0), stop=(idx == KTn * 4 - 1),
                            skip_group_check=True)

        def finalize_av(st):
            j, qh, avs, oj = st["j"], st["qh"], st["avs"], st["oj"]
            for pair in (0, 1):
                av = avs[pair]
                av3 = av[:].rearrange("p (x c) -> p x c", c=65)
                rc = nrm.tile([128, 4], F32, tag="rc",
                              name=f"rc{j}_{qh}_{pair}")
                nc.vector.reciprocal(
                    rc[:].rearrange("p (a b) -> p a b", b=1), av3[:, :, 64:65])
                for qi in (0, 1):
                    ql = pair * 2 + qi
                    qc = qh * 4 + ql
                    for hh in (0, 1):
                        i = qi * 2 + hh
                        nc.vector.tensor_scalar_mul(
                            oj[:, ql, hh * 64:hh * 64 + 64],
                            av[:, i * 65:i * 65 + 64], rc[:, i:i + 1])
                    nc.sync.dma_start_transpose(
                        OTs[j][:, qc * 128:(qc + 1) * 128], oj[:, ql, :])

        HIPRI = 1 << 20
        units = [(j, qh) for j in range(EC) for qh in (0, 1)]

        # Filler inventory: (emission_deadline_unit, closure). Paced pops
        # drain one item per two kt-steps; any item whose deadline arrives
        # is force-emitted at the top of that unit (emission must precede
        # the first reader -- dependency tracking is program-order
        # directional). List order is psF-execution order: m1 fillers
        # before the V block so attention unit 2 isn't gated on V-proj.
        fillers = [
            (2, lambda: q_pass(1, 1)),
            (2, lambda: k_slot(1)),
        ]
        for t in range(KTn):
            fillers.append((1, lambda t=t: v_chunk(t, 0)))
            fillers.append((1, lambda t=t: v_chunk(t, 1)))
        for m in range(2, EC):
            fillers.append((2 * m, lambda m=m: q_pass(m, 1)))
            fillers.append((2 * m, lambda m=m: k_slot(m)))
        oA = [lambda m=m, n0=n0: o_chunk(m, n0, 0, 4)
              for m in range(EC - 1) for n0 in (0, 512)]
        oB = [lambda m=m, n0=n0: o_chunk(m, n0, 4, 3)
              for m in range(EC - 1) for n0 in (0, 512)]
        oC = [lambda m=m, n0=n0: o_chunk(m, n0, 7, 1)
              for m in range(EC - 1) for n0 in (0, 512)]
        oC += [lambda n0=n0: o_chunk(EC - 1, n0, 0, EC) for n0 in (0, 512)]

        prev = None
        for u, (j, qh) in enumerate(units):
            ptiles = []
            if u == 9:
                fillers.extend((10 ** 9, c) for c in oA)
            if u == 15:
                fillers.extend((10 ** 9, c) for c in oB)
            due = [f for f in fillers if f[0] <= u]
            if due:
                fillers = [f for f in fillers if f[0] > u]
                for _, c in due:
                    c()
            if u == 4:
                # vt/wv no longer needed; reuse the SBUF for wo/yac/yt.
                inpV_ctx.close()
                wo_pool = ctx.enter_context(tc.tile_pool(name="wop", bufs=1))
                wo_t[0] = wo_pool.tile([128, EC, E], BF16, name="wo")
                wov = woT[:].rearrange("(c p) e -> p c e", p=128)
                nc.scalar.dma_start(wo_t[0][:, 0:4, :], wov[:, 0:4, :])
                nc.scalar.dma_start(wo_t[0][:, 4:8, :], wov[:, 4:8, :])
                yac.extend(wo_pool.tile([128, Q], BF16, name=f"yac{m}")
                           for m in range(EC - 1))
                yts.extend(wo_pool.tile([128, 512], BF16, name=f"yt{i}")
                           for i in range(2))
            for kt in range(KTn):
                with tc.high_priority(offset=HIPRI):
                    pe = psE.tile([128, 1024], F32, tag="e",
                                  name=f"pe{j}_{qh}_{kt}")
                    for hh in (0, 1):
                        off = hh * 64
                        nc.tensor.matmul(
                            pe[:, hh * 512:hh * 512 + 512],
                            KTs[j][off:off + 64, kt * 128:(kt + 1) * 128],
                            QTs[j][off:off + 64, qh * 512:qh * 512 + 512])
                    pt = pp.tile([128, 1024], BF16, tag=f"P{qh}_{kt}",
                                 name=f"pt{j}_{qh}_{kt}")
                    nc.scalar.activation(
                        pt[:], pe[:], mybir.ActivationFunctionType.Exp,
                        bias=mb_s[:, kt:kt + 1], scale=0.125)
                    ptiles.append(pt)
                    if prev is not None:
                        if kt == 0:
                            prev["avs"] = [
                                psA.tile([128, 260], F32, tag=f"a{pr}",
                                         name=f"av{prev['j']}_{prev['qh']}_{pr}")
                                for pr in (0, 1)]
                        emit_av_kt(prev, kt)
                if kt % 2 == 1 and fillers:
                    fillers.pop(0)[1]()
            if prev is not None:
                with tc.high_priority(offset=HIPRI):
                    finalize_av(prev)
            prev = dict(j=j, qh=qh, pt=ptiles, avs=None,
                        oj=nrm.tile([128, 4, 128], BF16, tag="oj",
                                    name=f"oj{j}_{qh}"))
        with tc.high_priority(offset=HIPRI):
            prev["avs"] = [psA.tile([128, 260], F32, tag=f"a{pr}",
                                    name=f"av_last_{pr}") for pr in (0, 1)]
            for kt in range(KTn):
                emit_av_kt(prev, kt)
            finalize_av(prev)
        while fillers:
            fillers.pop(0)[1]()
        for c in oC:
            c()

    nc.compile()
    return nc


_PROG_CACHE = {}


def _get_program(Kpad, ntrim):
    key = (Kpad, ntrim)
    if key not in _PROG_CACHE:
        _PROG_CACHE[key] = build_program(Kpad, ntrim)
    return _PROG_CACHE[key]


def prepare_inputs(query, keys, values, mask, Wq, bq, Wk, bk, Wv, bv, Wo, bo):
    """Host-side sharding/layout prep. Returns (Kpad, ntrim, in_maps)."""
    f32 = np.float32
    query = np.asarray(query, f32)
    keys = np.asarray(keys, f32)
    values = np.asarray(values, f32)
    mask = np.asarray(mask)

    idxs = [np.nonzero(mask[b] != 0)[0] for b in range(B)]
    nmax = max(len(i) for i in idxs)
    Kpad = max(256, ((max(nmax, 1) + 127) // 128) * 128)
    KTn = Kpad // 128
    ntrim = min(Kpad, ((max(nmax, 1) + 3) // 4) * 4)

    kTb = np.zeros((B, E, Kpad), BF16NP)
    vTb = np.zeros((B, E, Kpad), BF16NP)
    mbb = np.full((B, Kpad), -1e9, f32)
    for b in range(B):
        n = len(idxs[b])
        kTb[b, :, :n] = keys[b][idxs[b]].T.astype(BF16NP)
        vTb[b, :, :n] = values[b][idxs[b]].T.astype(BF16NP)
        mbb[b, :n] = 0.0
    mb2 = np.ascontiguousarray(mbb.reshape(B, KTn, 128).transpose(0, 2, 1))

    WqT = np.ascontiguousarray(np.asarray(Wq, f32).T.astype(BF16NP))
    WkT = np.ascontiguousarray(np.asarray(Wk, f32).T.astype(BF16NP))
    WvT = np.ascontiguousarray(np.asarray(Wv, f32).T.astype(BF16NP))
    WoT = np.ascontiguousarray(np.asarray(Wo, f32).T.astype(BF16NP))
    bq2 = np.ascontiguousarray(np.asarray(bq, f32).reshape(EC, 128).T)
    bk2 = np.ascontiguousarray(np.asarray(bk, f32).reshape(EC, 128).T)
    # fold V bias through the output projection: y += (Wo @ bv + bo)
    bo_f = np.asarray(bo, f32) + np.asarray(Wo, f32) @ np.asarray(bv, f32)
    bo2 = np.ascontiguousarray(bo_f.reshape(EC, 128).T)

    in_maps = []
    for c in range(NCORES):
        b, h = c // 2, c % 2
        in_maps.append(dict(
            qT=np.ascontiguousarray(
                query[b, h * Q:(h + 1) * Q, :].T.astype(BF16NP)),
            kT=kTb[b], vT=vTb[b], mb=mb2[b],
            wqT=WqT, wkT=WkT, wvT=WvT, woT=WoT,
            bq2=bq2, bk2=bk2, bo2=bo2,
        ))
    return Kpad, ntrim, in_maps


def kernel(query, keys, values, mask, Wq, bq, Wk, bk, Wv, bv, Wo, bo):
    Kpad, ntrim, in_maps = prepare_inputs(query, keys, values, mask,
                                          Wq, bq, Wk, bk, Wv, bv, Wo, bo)
    nc = _get_program(Kpad, ntrim)
    res = run_bass_kernel_spmd(nc, in_maps, list(range(NCORES)))
    out = np.empty((B, S, E), np.float32)
    for c in range(NCORES):
        b, h = c // 2, c % 2
        out[b, h * Q:(h + 1) * Q, :] = \
            res.results[c]["yT"].T.astype(np.float32)
    return out


# revision 33
# speedup vs baseline: 1.0256x; 1.0048x over previous
"""Multi-head attention on 8 TRN2 NeuronCores (Bass/Tile).

Sharding: core c handles batch b = c//2 and query-half h = c%2 (1024 query
tokens), all 16 heads. K/V projections are per-batch and duplicated across
the two cores sharing a batch; no cross-core communication (pair-AllGather
dedup was prototyped but the cost model prices a collective at 15us +
total_bytes/40GB/s, which never pays for the ~15-30us of PE it saves).

Design notes (v9, evolved from v3):
- All matmul operands bf16 (PSUM fp32). Keys compacted on host via the 0/1
  mask; pad keys killed by a -1e9 per-partition bias folded into Exp.
- V-proj bias folded into output bias host-side (bo' = bo + Wo @ bv).
- Energy computed transposed ([key, query] tiles); AV accumulates
  out[q, 65] per head with a ones column carrying the softmax denominator.
- K-projection free dims trimmed to the actual key count (ntrim); KT pad
  columns memset to zero. kt input tile also trimmed.
- Input DMAs split across the two HWDGE queues: SP carries qt/wq/kt/wk +
  transposes + output; the Activation queue carries vt/wv/wo. (The Pool
  SWDGE path was tried and CORRUPTS data under concurrent consumers --
  it races the cross-engine readers; the Act queue is a proper HWDGE
  path and is safe.)
- Out-projection split in three k-chunk groups per (m, n0): A=k0-3 (ready
  once OTs[0..3] final, after unit 9), B=k4-6 (after unit 15), C=k7
  (tail), accumulating into bf16 SBUF tiles (yac). m=7 runs entirely at
  the tail (its yac slot didn't fit in SBUF). Output yT is bf16 (host
  converts to f32); total extra error ~0.2-0.3% rms, well inside 2e-2.
- Attention phase is PE-bound (PE busy ~207us > Act exp stream ~151us).
  Q-proj runs as two 4-kk passes so pass 0 starts when the first qt/wq
  half lands. Phase 1 carries Q m0 per-kk + pass0 all m + pass1 m0 +
  K m0; the rest (Q pass1 m1-7, K m1-7, V t0-8, out-proj groups A/B)
  drain as a deadline-tagged filler deque, one chunk per two kt-steps;
  an item is force-emitted at the top of the unit that first reads its
  output (dependency tracking is program-order directional: a read
  emitted before its writer races it, which shows up as NaN on HW while
  the timeline simulator still passes).
- The PE p-state model punishes sparse matmul streams: a gap resets the
  ramp and subsequent matmuls run at half clock for ~3us. Prefer dense
  chunks (>=4 matmuls per psF tile, PE-bound vs the DVE consumer) over
  per-kk trickles.
"""

import sys

sys.path.insert(0, "/opt/trn_rl_repo")

from contextlib import ExitStack

import ml_dtypes
import numpy as np

import concourse.bass as bass  # noqa: F401
import concourse.tile as tile
from concourse import bacc, mybir
from concourse.bass_utils import run_bass_kernel_spmd

E = 1024          # embed dim
HEADS = 16
HD = 64           # head dim
B = 4
S = 2048
NCORES = 8
Q = (B * S) // NCORES  # query tokens per core
EC = E // 128     # embed chunks of 128
F32 = mybir.dt.float32
BF16 = mybir.dt.bfloat16
BF16NP = ml_dtypes.bfloat16


def build_program(Kpad, ntrim):
    """Per-core Bass program (identical on all 8 cores)."""
    KTn = Kpad // 128
    nc = bacc.Bacc("TRN2", target_bir_lowering=False, debug=False,
                   num_devices=NCORES, dynamic_dma_scratch_size=2048)

    qT = nc.dram_tensor("qT", [E, Q], BF16, kind="ExternalInput").ap()
    kT = nc.dram_tensor("kT", [E, Kpad], BF16, kind="ExternalInput").ap()
    vT = nc.dram_tensor("vT", [E, Kpad], BF16, kind="ExternalInput").ap()
    wqT = nc.dram_tensor("wqT", [E, E], BF16, kind="ExternalInput").ap()
    wkT = nc.dram_tensor("wkT", [E, E], BF16, kind="ExternalInput").ap()
    wvT = nc.dram_tensor("wvT", [E, E], BF16, kind="ExternalInput").ap()
    woT = nc.dram_tensor("woT", [E, E], BF16, kind="ExternalInput").ap()
    bq2 = nc.dram_tensor("bq2", [128, EC], F32, kind="ExternalInput").ap()
    bk2 = nc.dram_tensor("bk2", [128, EC], F32, kind="ExternalInput").ap()
    bo2 = nc.dram_tensor("bo2", [128, EC], F32, kind="ExternalInput").ap()
    mb = nc.dram_tensor("mb", [128, KTn], F32, kind="ExternalInput").ap()
    yT = nc.dram_tensor("yT", [E, Q], BF16, kind="ExternalOutput").ap()

    with tile.TileContext(nc) as tc, ExitStack() as ctx:
        sml = ctx.enter_context(tc.tile_pool(name="sml", bufs=1))
        big = ctx.enter_context(tc.tile_pool(name="big", bufs=1))

        # ---- PSUM pools: psE 2x[128,1024]=4 banks, psA 2x1=2, psF 2x1=2
        psE = ctx.enter_context(tc.tile_pool(name="psE", bufs=2, space="PSUM"))
        psA = ctx.enter_context(tc.tile_pool(name="psA", bufs=1, space="PSUM"))
        psF = ctx.enter_context(tc.tile_pool(name="psF", bufs=2, space="PSUM"))

        inp = ctx.enter_context(tc.tile_pool(name="inp", bufs=1))
        pp = ctx.enter_context(tc.tile_pool(name="pp", bufs=1))
        nrm = ctx.enter_context(tc.tile_pool(name="nrm", bufs=2))
        # vt/wv free mid-attention; inpV sits atop the pool stack so its
        # SBUF can be reused for wo/yac/yt.
        inpV_ctx = ExitStack()
        inpV = inpV_ctx.enter_context(tc.tile_pool(name="inpV", bufs=1))

        # ---- big input DMAs (single SP/HWDGE queue, consumption order) ---
        qt_t, wq_t = [], []
        qv = qT[:].rearrange("(c p) q -> p c q", p=128)
        wqv = wqT[:].rearrange("(c p) e -> p c e", p=128)
        bq_s = sml.tile([128, EC], F32, name="bq_s")
        bk_s = sml.tile([128, EC], F32, name="bk_s")
        bo_s = sml.tile([128, EC], F32, name="bo_s")
        mb_s = sml.tile([128, KTn], F32, name="mb_s")
        for h in (0, 1):
            t = inp.tile([128, 4, Q], BF16, name=f"qt{h}")
            w = inp.tile([128, 4, E], BF16, name=f"wq{h}")
            for kk in range(4):
                nc.sync.dma_start(t[:, kk, :], qv[:, 4 * h + kk, :])
                nc.sync.dma_start(w[:, kk, :], wqv[:, 4 * h + kk, :])
                if h == 0 and kk == 1:
                    nc.sync.dma_start(bq_s[:], bq2[:])
                    nc.sync.dma_start(bk_s[:], bk2[:])
                    nc.sync.dma_start(bo_s[:], bo2[:])
                    nc.sync.dma_start(mb_s[:], mb[:])
            qt_t.append(t)
            wq_t.append(w)
        kt_t = inp.tile([128, EC, ntrim], BF16, name="kt")
        kv4 = kT[:].rearrange("(c p) k -> p c k", p=128)
        nc.sync.dma_start(kt_t[:, 0:4, :], kv4[:, 0:4, 0:ntrim])
        nc.sync.dma_start(kt_t[:, 4:8, :], kv4[:, 4:8, 0:ntrim])
        wk_t = inp.tile([128, EC, E], BF16, name="wk")
        wkv = wkT[:].rearrange("(c p) e -> p c e", p=128)
        nc.sync.dma_start(wk_t[:, 0:4, :], wkv[:, 0:4, :])
        nc.sync.dma_start(wk_t[:, 4:8, :], wkv[:, 4:8, :])
        vt_t = inpV.tile([128, EC, Kpad], BF16, name="vt")
        vv4 = vT[:].rearrange("(c p) k -> p c k", p=128)
        wv_t = inpV.tile([128, EC, E], BF16, name="wv")
        wvv = wvT[:].rearrange("(c p) e -> p c e", p=128)
        for c0 in (0, 2, 4, 6):
            nc.scalar.dma_start(vt_t[:, c0:c0 + 2, :], vv4[:, c0:c0 + 2, :])
            nc.scalar.dma_start(wv_t[:, c0:c0 + 2, :], wvv[:, c0:c0 + 2, :])

        # ---- persistent SBUF tensors -------------------------------------
        QTs = [big.tile([128, Q], BF16, name=f"QT{m}") for m in range(EC)]
        KTs = [big.tile([128, Kpad], BF16, name=f"KT{m}") for m in range(EC)]
        VVs = [big.tile([128, HEADS * 65], BF16, name=f"VV{t}")
               for t in range(KTn)]
        OTs = [big.tile([128, Q], BF16, name=f"OT{m}") for m in range(EC)]
        # ones columns of the V tiles, written once before anything reads VV
        for t in range(KTn):
            vv3 = VVs[t][:].rearrange("p (h e) -> p h e", e=65)
            nc.vector.memset(vv3[:, :, 64:65], 1.0)

        # K-proj free-dim chunks, trimmed to ntrim
        kchunks = []
        for n0 in range(0, ntrim, 512):
            kchunks.append((n0, min(512, ntrim - n0)))

        # ---- projection emitters -----------------------------------------
        def q_pass(m, half):
            """Half-contraction Q-proj pass: kk in [half*4, half*4+4).
            Pass 0 writes QTs[m] (with bias); pass 1 accumulates."""
            for n0 in (0, 512):
                q_chunk(m, n0, half * 4, 4, half == 0)

        def q_chunk(m, n0, k0=0, nk=EC, first=True):
            ps = psF.tile([128, 512], F32, tag="f", name=f"psq{m}_{n0}_{k0}")
            for kk in range(k0, k0 + nk):
                nc.tensor.matmul(
                    ps[:], wq_t[kk // 4][:, kk % 4, m * 128:(m + 1) * 128],
                    qt_t[kk // 4][:, kk % 4, n0:n0 + 512],
                    start=(kk == k0), stop=(kk == k0 + nk - 1))
            if first:
                nc.vector.tensor_scalar_add(
                    QTs[m][:, n0:n0 + 512], ps[:], bq_s[:, m:m + 1])
            else:
                with nc.allow_low_precision(reason="bf16 proj accum"):
                    nc.vector.tensor_add(
                        QTs[m][:, n0:n0 + 512], QTs[m][:, n0:n0 + 512], ps[:])

        def k_chunk(m, n0, nn, last):
            ps = psF.tile([128, 512], F32, tag="f", name=f"psk{m}_{n0}")
            for kk in range(EC):
                nc.tensor.matmul(
                    ps[:, 0:nn], wk_t[:, kk, m * 128:(m + 1) * 128],
                    kt_t[:, kk, n0:n0 + nn],
                    start=(kk == 0), stop=(kk == EC - 1))
            nc.vector.tensor_scalar_add(
                KTs[m][:, n0:n0 + nn], ps[:, 0:nn], bk_s[:, m:m + 1])
            if last and ntrim < Kpad:
                nc.vector.memset(KTs[m][:, ntrim:Kpad], 0.0)

        def k_slot(m):
            for ci, (n0, nn) in enumerate(kchunks):
                k_chunk(m, n0, nn, ci == len(kchunks) - 1)

        def v_chunk(t, half):
            """V-proj chunk: heads half*8..half*8+8 of key tile t."""
            n0 = half * 512
            ps = psF.tile([128, 512], F32, tag="f", name=f"psv{t}_{half}")
            for kk in range(EC):
                nc.tensor.matmul(
                    ps[:], vt_t[:, kk, t * 128:(t + 1) * 128],
                    wv_t[:, kk, n0:n0 + 512],
                    start=(kk == 0), stop=(kk == EC - 1))
            vv3 = VVs[t][:].rearrange("p (h e) -> p h e", e=65)
            ps3 = ps[:].rearrange("p (h d) -> p h d", d=64)
            nc.vector.tensor_copy(vv3[:, half * 8:half * 8 + 8, 0:64], ps3[:])

        # ---- phase 1 emission -------------------------------------------
        # Q pass 0 (kk 0-3) for all m as the first qt/wq half arrives;
        # pass 1 for m=0, then K m0 -> attention unit 0 can start.
        for k in range(4):
            for n0 in (0, 512):
                q_chunk(0, n0, k, 1, k == 0)
        for m in range(1, EC):
            q_pass(m, 0)
        q_pass(0, 1)
        k_slot(0)

        # ---- out-projection emitters -------------------------------------
        wo_t = [None]
        yac = []   # bf16 accumulators for m=0..6, allocated in wop at u==4
        yts = []   # two rotating bf16 output staging tiles

        def o_chunk(m, n0, k0, nk):
            ps = psF.tile([128, 512], F32, tag="f", name=f"pso{m}_{n0}_{k0}")
            for k in range(k0, k0 + nk):
                nc.tensor.matmul(
                    ps[:], wo_t[0][:, k, m * 128:(m + 1) * 128],
                    OTs[k][:, n0:n0 + 512],
                    start=(k == k0), stop=(k == k0 + nk - 1))
            if k0 == 0 and nk < EC:
                with nc.allow_low_precision(reason="bf16 out-proj accum"):
                    nc.vector.tensor_copy(yac[m][:, n0:n0 + 512], ps[:])
            elif k0 + nk < EC:
                with nc.allow_low_precision(reason="bf16 out-proj accum"):
                    nc.vector.tensor_add(yac[m][:, n0:n0 + 512],
                                         yac[m][:, n0:n0 + 512], ps[:])
            elif nk == EC:  # m=7: whole contraction in one chunk
                yt = yts[n0 // 512]
                with nc.allow_low_precision(reason="bf16 output"):
                    nc.vector.tensor_scalar_add(
                        yt[:], ps[:], bo_s[:, m:m + 1])
                nc.sync.dma_start(yT[m * 128:(m + 1) * 128, n0:n0 + 512],
                                  yt[:])
            else:
                # finish in place: yac[m] slice becomes the output staging
                dst = yac[m][:, n0:n0 + 512]
                with nc.allow_low_precision(reason="bf16 output"):
                    nc.vector.scalar_tensor_tensor(
                        dst, ps[:], bo_s[:, m:m + 1], dst,
                        op0=mybir.AluOpType.add, op1=mybir.AluOpType.add)
                nc.sync.dma_start(yT[m * 128:(m + 1) * 128, n0:n0 + 512],
                                  dst)

        # ---- attention: units of (head pair j, query half qh) ------------
        def emit_av_kt(st, kt):
            j, ptiles, avs = st["j"], st["pt"], st["avs"]
            for pair in (0, 1):
                av = avs[pair]
                for qi in (0, 1):
                    for hh in (0, 1):
                        idx = kt * 4 + qi * 2 + hh
                        q0 = hh * 512 + (pair * 2 + qi) * 128
                        nc.tensor.matmul(
                            av[:, (qi * 2 + hh) * 65:(qi * 2 + hh + 1) * 65],
                            ptiles[kt][:, q0:q0 + 128],
                            VVs[kt][:, (2 * j + hh) * 65:(2 * j + hh + 1) * 65],
                            start=(idx == 0), stop=(idx == KTn * 4 - 1),
                            skip_group_check=True)

        def finalize_av(st):
            j, qh, avs, oj = st["j"], st["qh"], st["avs"], st["oj"]
            for pair in (0, 1):
                av = avs[pair]
                av3 = av[:].rearrange("p (x c) -> p x c", c=65)
                rc = nrm.tile([128, 4], F32, tag="rc",
                              name=f"rc{j}_{qh}_{pair}")
                nc.vector.reciprocal(
                    rc[:].rearrange("p (a b) -> p a b", b=1), av3[:, :, 64:65])
                for qi in (0, 1):
                    ql = pair * 2 + qi
                    qc = qh * 4 + ql
                    for hh in (0, 1):
                        i = qi * 2 + hh
                        nc.vector.tensor_scalar_mul(
                            oj[:, ql, hh * 64:hh * 64 + 64],
                            av[:, i * 65:i * 65 + 64], rc[:, i:i + 1])
                    nc.sync.dma_start_transpose(
                        OTs[j][:, qc * 128:(qc + 1) * 128], oj[:, ql, :])

        HIPRI = 1 << 20
        units = [(j, qh) for j in range(EC) for qh in (0, 1)]

        # Filler inventory: (emission_deadline_unit, closure). Paced pops
        # drain one item per two kt-steps; any item whose deadline arrives
        # is force-emitted at the top of that unit (emission must precede
        # the first reader -- dependency tracking is program-order
        # directional). List order is psF-execution order: m1 fillers
        # before the V block so attention unit 2 isn't gated on V-proj.
        def k_chunk_items(m):
            return [(2 * m, lambda m=m, n0=n0, nn=nn, last=last:
                     k_chunk(m, n0, nn, last))
                    for (n0, nn), last in
                    [(c, i == len(kchunks) - 1)
                     for i, c in enumerate(kchunks)]]

        def q_half_items(m):
            return [(2 * m, lambda m=m, n0=n0: q_chunk(m, n0, 4, 4, False))
                    for n0 in (0, 512)]

        fillers = q_half_items(1) + k_chunk_items(1)
        for t in range(KTn):
            fillers.append((1, lambda t=t: v_chunk(t, 0)))
            fillers.append((1, lambda t=t: v_chunk(t, 1)))
        for m in range(2, EC):
            fillers.extend(q_half_items(m))
            fillers.extend(k_chunk_items(m))
        oA = [lambda m=m, n0=n0: o_chunk(m, n0, 0, 4)
              for m in range(EC - 1) for n0 in (0, 512)]
        oB = [lambda m=m, n0=n0: o_chunk(m, n0, 4, 3)
              for m in range(EC - 1) for n0 in (0, 512)]
        oC = [lambda m=m, n0=n0: o_chunk(m, n0, 7, 1)
              for m in range(EC - 1) for n0 in (0, 512)]
        oC += [lambda n0=n0: o_chunk(EC - 1, n0, 0, EC) for n0 in (0, 512)]

        prev = None
        for u, (j, qh) in enumerate(units):
            ptiles = []
            if u == 9:
                fillers.extend((10 ** 9, c) for c in oA)
            if u == 15:
                fillers.extend((10 ** 9, c) for c in oB)
            due = [f for f in fillers if f[0] <= u]
            if due:
                fillers = [f for f in fillers if f[0] > u]
                for _, c in due:
                    c()
            if u == 4:
                # vt/wv no longer needed; reuse the SBUF for wo/yac/yt.
                inpV_ctx.close()
                wo_pool = ctx.enter_context(tc.tile_pool(name="wop", bufs=1))
                wo_t[0] = wo_pool.tile([128, EC, E], BF16, name="wo")
                wov = woT[:].rearrange("(c p) e -> p c e", p=128)
                nc.scalar.dma_start(wo_t[0][:, 0:4, :], wov[:, 0:4, :])
                nc.scalar.dma_start(wo_t[0][:, 4:8, :], wov[:, 4:8, :])
                yac.extend(wo_pool.tile([128, Q], BF16, name=f"yac{m}")
                           for m in range(EC - 1))
                yts.extend(wo_pool.tile([128, 512], BF16, name=f"yt{i}")
                           for i in range(2))
            for kt in range(KTn):
                with tc.high_priority(offset=HIPRI):
                    pe = psE.tile([128, 1024], F32, tag="e",
                                  name=f"pe{j}_{qh}_{kt}")
                    for hh in (0, 1):
                        off = hh * 64
                        nc.tensor.matmul(
                            pe[:, hh * 512:hh * 512 + 512],
                            KTs[j][off:off + 64, kt * 128:(kt + 1) * 128],
                            QTs[j][off:off + 64, qh * 512:qh * 512 + 512])
                    pt = pp.tile([128, 1024], BF16, tag=f"P{qh}_{kt}",
                                 name=f"pt{j}_{qh}_{kt}")
                    nc.scalar.activation(
                        pt[:], pe[:], mybir.ActivationFunctionType.Exp,
                        bias=mb_s[:, kt:kt + 1], scale=0.125)
                    ptiles.append(pt)
                    if prev is not None:
                        if kt == 0:
                            prev["avs"] = [
                                psA.tile([128, 260], F32, tag=f"a{pr}",
                                         name=f"av{prev['j']}_{prev['qh']}_{pr}")
                                for pr in (0, 1)]
                        emit_av_kt(prev, kt)
                if kt % 2 == 1 and fillers:
                    fillers.pop(0)[1]()
            if prev is not None:
                with tc.high_priority(offset=HIPRI):
                    finalize_av(prev)
            prev = dict(j=j, qh=qh, pt=ptiles, avs=None,
                        oj=nrm.tile([128, 4, 128], BF16, tag="oj",
                                    name=f"oj{j}_{qh}"))
        with tc.high_priority(offset=HIPRI):
            prev["avs"] = [psA.tile([128, 260], F32, tag=f"a{pr}",
                                    name=f"av_last_{pr}") for pr in (0, 1)]
            for kt in range(KTn):
                emit_av_kt(prev, kt)
            finalize_av(prev)
        while fillers:
            fillers.pop(0)[1]()
        for c in oC:
            c()

    nc.compile()
    return nc


_PROG_CACHE = {}


def _get_program(Kpad, ntrim):
    key = (Kpad, ntrim)
    if key not in _PROG_CACHE:
        _PROG_CACHE[key] = build_program(Kpad, ntrim)
    return _PROG_CACHE[key]


def prepare_inputs(query, keys, values, mask, Wq, bq, Wk, bk, Wv, bv, Wo, bo):
    """Host-side sharding/layout prep. Returns (Kpad, ntrim, in_maps)."""
    f32 = np.float32
    query = np.asarray(query, f32)
    keys = np.asarray(keys, f32)
    values = np.asarray(values, f32)
    mask = np.asarray(mask)

    idxs = [np.nonzero(mask[b] != 0)[0] for b in range(B)]
    nmax = max(len(i) for i in idxs)
    Kpad = max(256, ((max(nmax, 1) + 127) // 128) * 128)
    KTn = Kpad // 128
    ntrim = min(Kpad, ((max(nmax, 1) + 3) // 4) * 4)

    kTb = np.zeros((B, E, Kpad), BF16NP)
    vTb = np.zeros((B, E, Kpad), BF16NP)
    mbb = np.full((B, Kpad), -1e9, f32)
    for b in range(B):
        n = len(idxs[b])
        kTb[b, :, :n] = keys[b][idxs[b]].T.astype(BF16NP)
        vTb[b, :, :n] = values[b][idxs[b]].T.astype(BF16NP)
        mbb[b, :n] = 0.0
    mb2 = np.ascontiguousarray(mbb.reshape(B, KTn, 128).transpose(0, 2, 1))

    WqT = np.ascontiguousarray(np.asarray(Wq, f32).T.astype(BF16NP))
    WkT = np.ascontiguousarray(np.asarray(Wk, f32).T.astype(BF16NP))
    WvT = np.ascontiguousarray(np.asarray(Wv, f32).T.astype(BF16NP))
    WoT = np.ascontiguousarray(np.asarray(Wo, f32).T.astype(BF16NP))
    bq2 = np.ascontiguousarray(np.asarray(bq, f32).reshape(EC, 128).T)
    bk2 = np.ascontiguousarray(np.asarray(bk, f32).reshape(EC, 128).T)
    # fold V bias through the output projection: y += (Wo @ bv + bo)
    bo_f = np.asarray(bo, f32) + np.asarray(Wo, f32) @ np.asarray(bv, f32)
    bo2 = np.ascontiguousarray(bo_f.reshape(EC, 128).T)

    in_maps = []
    for c in range(NCORES):
        b, h = c // 2, c % 2
        in_maps.append(dict(
            qT=np.ascontiguousarray(
                query[b, h * Q:(h + 1) * Q, :].T.astype(BF16NP)),
            kT=kTb[b], vT=vTb[b], mb=mb2[b],
            wqT=WqT, wkT=WkT, wvT=WvT, woT=WoT,
            bq2=bq2, bk2=bk2, bo2=bo2,
        ))
    return Kpad, ntrim, in_maps


def kernel(query, keys, values, mask, Wq, bq, Wk, bk, Wv, bv, Wo, bo):
    Kpad, ntrim, in_maps = prepare_inputs(query, keys, values, mask,
                                          Wq, bq, Wk, bk, Wv, bv, Wo, bo)
    nc = _get_program(Kpad, ntrim)
    res = run_bass_kernel_spmd(nc, in_maps, list(range(NCORES)))
    out = np.empty((B, S, E), np.float32)
    for c in range(NCORES):
        b, h = c // 2, c % 2
        out[b, h * Q:(h + 1) * Q, :] = \
            res.results[c]["yT"].T.astype(np.float32)
    return out
